# revision 23
# baseline (speedup 1.0000x reference)
"""3-layer GAT on 8 trn2 NeuronCores.

Strategy (graph/data parallel per sharding hint):
  - Nodes assigned to 8 cores x 49 blocks x 128 slots (degree-balanced LPT
    packing) -> permuted node order; table row = core*6272 + block*128 + slot.
  - Per layer, per node block: transform with rhs = [W | W@as | W@ad] (alpha
    terms folded into the matmul) -> bf16 table shard [6272, 384]; one
    AllGather (Shared output) per layer -> full table on every core.
  - Aggregation processes PAIRS of dst blocks: non-self edges of both blocks
    (dst-sorted) share one dma_gather per int16-index window (lo rows
    [0,32768), hi rows [17408,50176)), amortizing the Q7 descriptor-gen
    fixed cost -- the gather stream is the kernel's critical path.  One-hot
    scatter matrices M [edge,dst] / MT [dst,edge] are built on-device per
    block over its (static) tile range and feed matmuls for the per-edge ad
    term and the fused (feature | denom) accumulation in PSUM.  Self-loop
    contributions are computed from the local shard and never gathered.
  - Softmax max-shift skipped (logits O(1), exp safe; mathematically equal).
  - Next layer's transform is interleaved into the aggregation loop
    (block-level pipeline); layer 2 reduces via a mask matmul; final mean +
    linear head on host.  ACT engine runs only Exp; casts/copies/lrelu/relu
    are DVE ops (ACT copy truncates f32->bf16; DVE rounds).
"""

import os
import numpy as np
import ml_dtypes

# ---------------- problem constants (must match reference) ----------------
N = 50000
E = 800000
IN_C = 128
HID = 64
HEADS = 4
OUT_C = 64
F1 = HEADS * HID  # 256

# ---------------- sharding geometry ----------------
NCORES = 8
NB = 49            # dst blocks per core
BS = 128           # dst slots per block
NPC = NB * BS      # 6272 nodes per core
RTOT = NCORES * NPC  # 50176 table rows
KE_CAP = 1152      # lo/hi packing capacity per (block, kind)
LO_LIM = 32768     # lo window rows [0, 32768)
HI_BASE = 17408    # hi window rows [17408, 50176)
# Aggregation unit: PAIR_BLOCKS=2 shares one dma_gather per window between
# two dst blocks (fewer Q7 fixed costs, but bigger gathers stall the SWDGE
# ring); PAIR_BLOCKS=1 gathers per block (measured faster on HW).
PAIR_BLOCKS = int(os.environ.get("GAT_PAIR", "1"))
NPAIR = (NB + PAIR_BLOCKS - 1) // PAIR_BLOCKS

EL01 = 384         # table elems/row layers 0/1: 256 h + 4 as + 4 ad + pad
EL2 = 128          # table elems/row layer 2: 64 h + 1 as + 1 ad + pad
BF = ml_dtypes.bfloat16

GBUFS = 6          # gather tile double-buffer depth
SINGLE_PACKET = os.environ.get("GAT_SP", "0") == "1"


def _a16(x):
    return (int(x) + 15) // 16 * 16


# ---------------- host preprocessing ----------------

def preprocess(edge_index):
    """Node->(core,block,slot) assignment, pair-packed edge arrays, and the
    static pair geometry consumed by build_kernel."""
    import heapq

    e0 = np.asarray(edge_index[0], np.int64)
    e1 = np.asarray(edge_index[1], np.int64)
    nonself = e0 != e1
    src = e0[nonself]
    dst = e1[nonself]
    # self-edge multiplicity: 1 (PyG added loop) + natural self edges
    mult = np.ones(N, np.int64)
    np.add.at(mult, e0[~nonself], 1)

    deg = np.bincount(dst, minlength=N)  # gather load per dst node

    nblocks = NCORES * NB
    order = np.argsort(-deg, kind="stable")
    heap = [(0, b) for b in range(nblocks)]
    heapq.heapify(heap)
    slots_used = np.zeros(nblocks, np.int64)
    node_block = np.empty(N, np.int64)
    node_slot = np.empty(N, np.int64)
    for n in order:
        while True:
            load, b = heapq.heappop(heap)
            if slots_used[b] < BS:
                break
        node_block[n] = b
        node_slot[n] = slots_used[b]
        slots_used[b] += 1
        heapq.heappush(heap, (load + int(deg[n]), b))

    row = node_block * BS + node_slot  # block-major global table row

    xperm = np.full(RTOT, -1, np.int64)
    xperm[row] = np.arange(N)

    erow = row[src]
    eblk = node_block[dst]
    eslot = node_slot[dst]

    order_e = np.argsort(eblk, kind="stable")
    bounds = np.searchsorted(eblk[order_e], np.arange(nblocks + 1))

    # pass 1: split lo/hi per block, count
    packed = {}
    cnt = np.zeros((NCORES, NB, 2), np.int64)
    for b in range(nblocks):
        c, bl = divmod(b, NB)
        es = order_e[bounds[b]:bounds[b + 1]]
        r_ = erow[es]
        dl = eslot[es]
        lo_f = r_ < HI_BASE
        hi_f = r_ >= LO_LIM
        flex = ~lo_f & ~hi_f
        n_lo = int(lo_f.sum())
        n_hi = int(hi_f.sum())
        n_fx = int(flex.sum())
        tot = n_lo + n_hi + n_fx
        assert tot <= 2 * KE_CAP, f"block {b} has {tot} edges > {2*KE_CAP}"
        add_lo = min(n_fx, max(0, min(KE_CAP, (tot + 1) // 2) - n_lo))
        if n_hi + (n_fx - add_lo) > KE_CAP:
            add_lo = n_fx - (KE_CAP - n_hi)
        assert 0 <= add_lo <= n_fx
        fx_idx = np.nonzero(flex)[0]
        sel_lo = np.zeros(len(es), bool)
        sel_lo[lo_f] = True
        sel_lo[fx_idx[:add_lo]] = True
        for kind, sel, base in ((0, sel_lo, 0), (1, ~sel_lo, HI_BASE)):
            rr = r_[sel]
            dd = dl[sel]
            o = np.argsort(rr, kind="stable")  # DMA locality
            packed[(b, kind)] = (rr[o] - base, dd[o])
            cnt[c, bl, kind] = len(rr)

    # ---- static pair geometry ----
    # pair p = blocks (2p, 2p+1); per (pair, kind): section sizes sa/sb
    # (16-aligned max over cores), gather reg = sa+sb, tiles tk.
    # Per block and kind: tile range [t0, t1) within the pair's tiles.
    geo = []
    for p in range(NPAIR):
        ba = PAIR_BLOCKS * p
        bb = ba + 1
        has_b = PAIR_BLOCKS == 2 and bb < NB
        pk = []
        for kind in range(2):
            sa = _a16(cnt[:, ba, kind].max())
            sb = _a16(cnt[:, bb, kind].max()) if has_b else 0
            skp = sa + sb
            tk = (skp + 127) // 128
            a0, a1 = 0, (sa + 127) // 128
            b0, b1 = sa // 128, tk
            pk.append(dict(sa=sa, sb=sb, skp=skp, tk=tk,
                           a=(a0, a1), b=(b0, b1) if has_b else None))
        geo.append(pk)

    # flat column offsets for idx16 / per-block dstc (dstcX: per block, both
    # kinds adjacent: [lo tiles | hi tiles] over that block's tile ranges)
    idx_off = []
    o = 0
    for p in range(NPAIR):
        idx_off.append(o)
        o += (geo[p][0]["tk"] * 128 + geo[p][1]["tk"] * 128) // 16
    idx_cols = o

    blk_off = []   # per block: column offset into dstc flat (tile units)
    blk_nt = []    # per block: (nt_lo, nt_hi)
    o = 0
    for bl in range(NB):
        p, x = divmod(bl, PAIR_BLOCKS)
        rng = [geo[p][k]["a" if x == 0 else "b"] for k in range(2)]
        nt = [r[1] - r[0] for r in rng]
        blk_off.append(o)
        blk_nt.append(tuple(nt))
        o += nt[0] + nt[1]
    dstc_cols = o

    dstr_cols = max((blk_nt[b][0] + blk_nt[b][1]) * 128 for b in range(NB))
    idx16 = np.full((NCORES, 128, idx_cols), -1, np.int16)
    dstc = np.full((NCORES, 128, dstc_cols), -1.0, np.float32)
    dstr = np.full((NCORES, NB, dstr_cols), -1.0, np.float32)
    maskc = np.zeros((NCORES, 128, NB), np.float32)
    mselfc = np.zeros((NCORES, 128, NB), np.float32)

    for c in range(NCORES):
        for p in range(NPAIR):
            ba = PAIR_BLOCKS * p
            bb = ba + 1
            has_b = PAIR_BLOCKS == 2 and bb < NB
            col0 = idx_off[p]
            for kind in range(2):
                gk = geo[p][kind]
                sa, sb, skp, tk = gk["sa"], gk["sb"], gk["skp"], gk["tk"]
                kep = tk * 128
                relA, ddA = packed[(c * NB + ba, kind)]
                full = np.full(kep, -1, np.int64)
                dloc = np.full(kep, -1.0, np.float32)
                kA = len(relA)
                full[:kA] = relA
                full[kA:sa] = 0
                dloc[:kA] = ddA.astype(np.float32)
                if has_b:
                    relB, ddB = packed[(c * NB + bb, kind)]
                    kB = len(relB)
                    full[sa:sa + kB] = relB
                    full[sa + kB:skp] = 0
                    dloc[sa:sa + kB] = ddB.astype(np.float32)
                w = full.reshape(kep // 16, 16).T.astype(np.int16)
                idx16[c, :, col0:col0 + kep // 16] = np.tile(w, (8, 1))
                col0 += kep // 16
                # per-block dstc over tile ranges (other block's slots = -1)
                for x, blk in ((0, ba),) + (((1, bb),) if has_b else ()):
                    t0, t1 = gk["a"] if x == 0 else gk["b"]
                    dl = dloc.copy()
                    if x == 0:
                        dl[sa:] = -1.0
                    else:
                        dl[:sa] = -1.0
                    seg = dl.reshape(tk, 128)[t0:t1]          # [nt, 128]
                    off = blk_off[blk] + (0 if kind == 0 else blk_nt[blk][0])
                    nt = t1 - t0
                    dstc[c, :, off:off + nt] = seg.T
                    dstr[c, blk, (0 if kind == 0 else
                                  blk_nt[blk][0] * 128):][:nt * 128] = \
                        seg.reshape(-1)

        for bl in range(NB):
            b = c * NB + bl
            used = slots_used[b]
            maskc[c, :used, bl] = 1.0
            nodes_b = np.where(node_block == b)[0]
            mselfc[c, node_slot[nodes_b], bl] = \
                mult[nodes_b].astype(np.float32)

    return dict(row=row, xperm=xperm, idx16=idx16, dstc=dstc, dstr=dstr,
                maskc=maskc, mselfc=mselfc, cnt=cnt, geo=geo,
                idx_off=idx_off, blk_off=blk_off, blk_nt=blk_nt,
                node_block=node_block, node_slot=node_slot)


def host_weights(inputs):
    """Extended weight matrices with folded attention vectors."""
    def ext(W, a_s, a_d, heads):
        Wh = W.reshape(W.shape[0], heads, HID)
        Was = np.einsum("khc,hc->kh", Wh, a_s)
        Wad = np.einsum("khc,hc->kh", Wh, a_d)
        return np.concatenate([W, Was, Wad], axis=1).astype(np.float32)

    W0e = ext(np.asarray(inputs["W0"], np.float32),
              np.asarray(inputs["a0s"], np.float32),
              np.asarray(inputs["a0d"], np.float32), HEADS)      # [128, 264]
    W1e = ext(np.asarray(inputs["W1"], np.float32),
              np.asarray(inputs["a1s"], np.float32),
              np.asarray(inputs["a1d"], np.float32), HEADS)      # [256, 264]
    W2e = ext(np.asarray(inputs["W2"], np.float32),
              np.asarray(inputs["a2s"], np.float32),
              np.asarray(inputs["a2d"], np.float32), 1)          # [256, 66]
    return W0e, W1e, W2e


def build_core_inputs(inputs, pp):
    """Per-core in_maps for run_bass_kernel_spmd."""
    x = np.asarray(inputs["x"], np.float32)
    W0e, W1e, W2e = host_weights(inputs)
    b0 = np.asarray(inputs["b0"], np.float32)
    b1 = np.asarray(inputs["b1"], np.float32)
    b2 = np.asarray(inputs["b2"], np.float32)

    iota_row = np.tile(np.arange(128, dtype=np.float32), (128, 1))
    iota_col = np.arange(128, dtype=np.float32).reshape(128, 1)
    ones1 = np.ones((1, 128), np.float32)
    ident = np.eye(128, dtype=np.float32)

    consts = dict(
        w0e=W0e.astype(BF),
        w1e=W1e.reshape(2, 128, F1 + 2 * HEADS).astype(BF),
        w2e=W2e.reshape(2, 128, HID + 2).astype(BF),
        b0r=np.tile(b0, (128, 1)).astype(BF),
        b1r=np.tile(b1, (128, 1)).astype(BF),
        b2r=np.tile(b2, (128, 1)).astype(BF),
        iota_row=iota_row.astype(BF), iota_col=iota_col.astype(BF),
        ones1=ones1.astype(BF), ident=ident.astype(BF),
    )

    in_maps = []
    for c in range(NCORES):
        xtb = np.zeros((NB, IN_C, BS), np.float32)
        rows = np.arange(c * NPC, (c + 1) * NPC)
        nodes = pp["xperm"][rows].reshape(NB, BS)
        for b in range(NB):
            nb = nodes[b]
            valid = nb >= 0
            if valid.any():
                xtb[b][:, valid] = x[nb[valid]].T
        m = dict(
            xtb=xtb.astype(BF),
            idx16=pp["idx16"][c],
            dstc=pp["dstc"][c].astype(BF),
            dstr=pp["dstr"][c].astype(BF),
            maskc=pp["maskc"][c].astype(BF),
            mselfc=pp["mselfc"][c].astype(BF),
            **consts,
        )
        in_maps.append(m)
    return in_maps


# ---------------- numpy emulation of the device data path ----------------

def _emulate_layer(tables_in, pp, We, bias, heads, Fo, relu, el):
    """tables_in: node-major feature mat [RTOT, F_in] (f32).
    Mirrors the pair-packed device data path (reads pp's flat arrays)."""
    geo = pp["geo"]
    ncols = Fo + 2 * heads
    tb = (tables_in.astype(BF).astype(np.float32)
          @ We.astype(BF).astype(np.float32))
    table = np.zeros((RTOT, el), BF)
    table[:, :ncols] = tb.astype(BF)
    as_all = tb[:, Fo:Fo + heads].astype(BF).astype(np.float32)
    ad_all = tb[:, Fo + heads:Fo + 2 * heads].astype(BF).astype(np.float32)

    def lrexp(z):
        return np.exp(np.maximum(z, 0.2 * z)).astype(np.float32)

    out = np.zeros((RTOT, Fo), np.float32)
    for c in range(NCORES):
        for p in range(len(geo)):
            col0 = pp["idx_off"][p]
            gs = []
            dls = []
            for kind in range(2):
                gk = geo[p][kind]
                kep = gk["tk"] * 128
                w = pp["idx16"][c][:16, col0:col0 + kep // 16]
                col0 += kep // 16
                rel = w.T.reshape(-1).astype(np.int64)
                r = gk["skp"]
                base = 0 if kind == 0 else HI_BASE
                rows = rel[:r] + base
                g = np.zeros((kep, el), np.float32)
                g[:r] = np.asarray(table[rows], np.float32)
                gs.append(g)
            ba = PAIR_BLOCKS * p
            blks = ((0, ba),) + (((1, ba + 1),)
                                 if PAIR_BLOCKS == 2 and ba + 1 < NB else ())
            for x, blk in blks:
                bl = blk
                rbase = c * NPC + bl * BS
                agg = np.zeros((BS, Fo), np.float32)
                den = np.zeros((BS, heads), np.float32)
                for kind in range(2):
                    gk = geo[p][kind]
                    t0, t1 = gk["a"] if x == 0 else gk["b"]
                    off = pp["blk_off"][bl] + (
                        0 if kind == 0 else pp["blk_nt"][bl][0])
                    nt = t1 - t0
                    dl = pp["dstc"][c][:, off:off + nt].T.reshape(-1)
                    dl = dl.astype(np.int64)                    # [nt*128]
                    g = gs[kind][t0 * 128:t1 * 128]             # [nt*128, el]
                    valid = dl >= 0
                    a_s = g[:, Fo:Fo + heads]
                    a_d = np.where(valid[:, None],
                                   ad_all[rbase + dl % 128 if False else
                                          rbase + np.clip(dl, 0, None)], 0.0)
                    s = lrexp(a_s + a_d).astype(BF).astype(np.float32)
                    s = np.where(valid[:, None], s, 0.0)
                    hsc = (g[:, :Fo].reshape(-1, heads, HID)
                           * s[:, :, None]).astype(BF).astype(np.float32)
                    hsc = hsc.reshape(-1, Fo)
                    np.add.at(agg, np.clip(dl, 0, None)[valid], hsc[valid])
                    np.add.at(den, np.clip(dl, 0, None)[valid], s[valid])
                # self loops
                ms = pp["mselfc"][c][:, bl]
                ss = lrexp(as_all[rbase:rbase + BS]
                           + ad_all[rbase:rbase + BS])
                se = (ss * ms[:, None]).astype(BF).astype(np.float32)
                h_own = np.asarray(table[rbase:rbase + BS, :Fo], np.float32)
                hs = (h_own.reshape(BS, heads, HID)
                      * se[:, :, None]).astype(BF).astype(np.float32)
                agg += hs.reshape(BS, Fo)
                den += se
                o = agg.reshape(BS, heads, HID) / (den + 1e-16)[:, :, None]
                o = o.reshape(BS, Fo) + bias
                if relu:
                    o = np.maximum(o, 0.0)
                out[rbase:rbase + BS] = o
    return out


def emulate(inputs, pp=None):
    """Full numpy emulation; returns [1, OUT_C]."""
    if pp is None:
        pp = preprocess(np.asarray(inputs["edge_index"]))
    x = np.asarray(inputs["x"], np.float32)
    W0e, W1e, W2e = host_weights(inputs)
    h = np.zeros((RTOT, IN_C), np.float32)
    valid = pp["xperm"] >= 0
    h[valid] = x[pp["xperm"][valid]]

    b0 = np.asarray(inputs["b0"], np.float32)
    b1 = np.asarray(inputs["b1"], np.float32)
    b2 = np.asarray(inputs["b2"], np.float32)

    h0 = _emulate_layer(h, pp, W0e, b0, HEADS, F1, True, EL01)
    h1 = _emulate_layer(h0, pp, W1e, b1, HEADS, F1, True, EL01)
    h2 = _emulate_layer(h1, pp, W2e, b2, 1, HID, False, EL2)

    g = h2[valid].sum(axis=0, keepdims=True) / N
    return (g @ np.asarray(inputs["hw"], np.float32)
            + np.asarray(inputs["hb"], np.float32)).astype(np.float32)


# ---------------- device kernel ----------------

_BUILT = None
_BUILT_KEY = None


def build_kernel(geo, idx_off, blk_off, blk_nt, idx_cols, dstc_cols,
                 dstr_cols):
    import concourse.bacc as bacc
    import concourse.mybir as mybir
    import concourse.tile as tile
    from concourse import library_config

    f32 = mybir.dt.float32
    bf16 = mybir.dt.bfloat16
    i16 = mybir.dt.int16
    Alu = mybir.AluOpType
    Act = mybir.ActivationFunctionType

    nc = bacc.Bacc("TRN2", target_bir_lowering=False, debug=False,
                   num_devices=NCORES)

    max_tk = max(g[k]["tk"] for g in geo for k in range(2))
    max_nt = max(nt[0] + nt[1] for nt in blk_nt)

    # ---- I/O ----
    xtb_d = nc.dram_tensor("xtb", [NB, IN_C, BS], bf16, kind="ExternalInput")
    idx16_d = nc.dram_tensor("idx16", [128, idx_cols], i16,
                             kind="ExternalInput")
    dstc_d = nc.dram_tensor("dstc", [128, dstc_cols], bf16,
                            kind="ExternalInput")
    dstr_d = nc.dram_tensor("dstr", [NB, dstr_cols], bf16,
                            kind="ExternalInput")
    maskc_d = nc.dram_tensor("maskc", [128, NB], bf16, kind="ExternalInput")
    mselfc_d = nc.dram_tensor("mselfc", [128, NB], bf16, kind="ExternalInput")
    w0e_d = nc.dram_tensor("w0e", [IN_C, F1 + 2 * HEADS], bf16,
                           kind="ExternalInput")
    w1e_d = nc.dram_tensor("w1e", [2, 128, F1 + 2 * HEADS], bf16,
                           kind="ExternalInput")
    w2e_d = nc.dram_tensor("w2e", [2, 128, HID + 2], bf16,
                           kind="ExternalInput")
    b0r_d = nc.dram_tensor("b0r", [128, F1], bf16, kind="ExternalInput")
    b1r_d = nc.dram_tensor("b1r", [128, F1], bf16, kind="ExternalInput")
    b2r_d = nc.dram_tensor("b2r", [128, HID], bf16, kind="ExternalInput")
    iota_row_d = nc.dram_tensor("iota_row", [128, 128], bf16,
                                kind="ExternalInput")
    iota_col_d = nc.dram_tensor("iota_col", [128, 1], bf16,
                                kind="ExternalInput")
    ones1_d = nc.dram_tensor("ones1", [1, 128], bf16, kind="ExternalInput")
    ident_d = nc.dram_tensor("ident", [128, 128], bf16, kind="ExternalInput")
    out_d = nc.dram_tensor("out_part", [1, OUT_C], f32, kind="ExternalOutput")

    # internal DRAM
    tables = []
    shards = []
    for li, el in enumerate([EL01, EL01, EL2]):
        tables.append(nc.dram_tensor(f"table{li}", [RTOT, el], bf16,
                                     addr_space="Shared"))
        shards.append(nc.dram_tensor(f"shard{li}", [NPC, el], bf16))

    rg = [list(range(NCORES))]

    with tile.TileContext(nc) as tc:
        with (
            tc.tile_pool(name="const", bufs=1) as cpool,
            tc.tile_pool(name="gather", bufs=GBUFS) as gpool,
            tc.tile_pool(name="onehot", bufs=4) as mpool,
            tc.tile_pool(name="work", bufs=3) as wpool,
            tc.tile_pool(name="small", bufs=4) as spool,
            tc.tile_pool(name="adas", bufs=1) as apool,
            tc.tile_pool(name="ps_agg", bufs=2, space="PSUM") as ppagg,
            tc.tile_pool(name="ps_pad", bufs=2, space="PSUM") as pppad,
            tc.tile_pool(name="ps_rep", bufs=1, space="PSUM") as pprep,
            tc.tile_pool(name="ps_tp", bufs=1, space="PSUM") as pptp,
            tc.tile_pool(name="ps_tf", bufs=1, space="PSUM") as pptf,
            tc.tile_pool(name="ps_sum", bufs=1, space="PSUM") as ppsum,
        ):
            def load_const(tag, dram, shape, dtype=bf16, view=None):
                t = cpool.tile(shape, dtype, tag=tag)
                nc.sync.dma_start(out=t[:], in_=view if view is not None
                                  else dram[:])
                return t

            w0e_s = load_const("w0e", w0e_d, [IN_C, F1 + 2 * HEADS])
            w1e_s = load_const("w1e", w1e_d, [128, 2, F1 + 2 * HEADS],
                               view=w1e_d[:].rearrange("c p j -> p c j"))
            w2e_s = load_const("w2e", w2e_d, [128, 2, HID + 2],
                               view=w2e_d[:].rearrange("c p j -> p c j"))
            b0r_s = load_const("b0r", b0r_d, [128, F1])
            b1r_s = load_const("b1r", b1r_d, [128, F1])
            b2r_s = load_const("b2r", b2r_d, [128, HID])
            iota_row_s = load_const("iota_row", iota_row_d, [128, 128])
            iota_col_s = load_const("iota_col", iota_col_d, [128, 1])
            ones1_s = load_const("ones1", ones1_d, [1, 128])
            ident_s = load_const("ident", ident_d, [128, 128])
            idx16_s = load_const("idx16", idx16_d, [128, idx_cols], i16)
            dstc_s = load_const("dstc", dstc_d, [128, dstc_cols])
            maskc_s = load_const("maskc", maskc_d, [128, NB])
            mselfc_s = load_const("mselfc", mselfc_d, [128, NB])

            nc.gpsimd.load_library(library_config.mlp)

            # persistent per-layer alpha tiles [128, NB*heads]
            as_all0 = apool.tile([128, NB * HEADS], bf16, tag="as0")
            as_all1 = apool.tile([128, NB * HEADS], bf16, tag="as1")
            as_all2 = apool.tile([128, NB], bf16, tag="as2")
            ad_all0 = apool.tile([128, NB * HEADS], bf16, tag="ad0")
            ad_all1 = apool.tile([128, NB * HEADS], bf16, tag="ad1")
            ad_all2 = apool.tile([128, NB], bf16, tag="ad2")
            as_all = [as_all0, as_all1, as_all2]
            ad_all = [ad_all0, ad_all1, ad_all2]

            LCFG = [  # heads, Fo, ncols, el, bias, relu
                (HEADS, F1, F1 + 2 * HEADS, EL01, b0r_s, True),
                (HEADS, F1, F1 + 2 * HEADS, EL01, b1r_s, True),
                (1, HID, HID + 2, EL2, b2r_s, False),
            ]

            def transform_block(layer, b, lhsT0, lhsT1):
                heads, Fo, ncols, el, _bias, _relu = LCFG[layer]
                shard = shards[layer]
                ps = pptf.tile([128, 512], f32, tag="tf", space="PSUM")
                if layer == 0:
                    nc.tensor.matmul(out=ps[:, :ncols], lhsT=lhsT0,
                                     rhs=w0e_s[:], start=True, stop=True)
                else:
                    we = w1e_s if layer == 1 else w2e_s
                    nc.tensor.matmul(out=ps[:, :ncols], lhsT=lhsT0,
                                     rhs=we[:, 0, :], start=True, stop=False)
                    nc.tensor.matmul(out=ps[:, :ncols], lhsT=lhsT1,
                                     rhs=we[:, 1, :], start=False, stop=True)
                tb = wpool.tile([128, EL01], bf16, tag="tb")
                nc.vector.tensor_copy(out=tb[:, :ncols], in_=ps[:, :ncols])
                nc.vector.tensor_copy(
                    out=as_all[layer][:, b * heads:(b + 1) * heads],
                    in_=ps[:, Fo:Fo + heads])
                nc.vector.tensor_copy(
                    out=ad_all[layer][:, b * heads:(b + 1) * heads],
                    in_=ps[:, Fo + heads:Fo + 2 * heads])
                nc.sync.dma_start(out=shard[b * BS:(b + 1) * BS, :],
                                  in_=tb[:, :el])

            def allgather(layer):
                nc.gpsimd.collective_compute(
                    "AllGather", mybir.AluOpType.bypass,
                    replica_groups=rg, ins=[shards[layer][:].opt()],
                    outs=[tables[layer][:].opt()])

            def agg_pair(layer, p):
                """Aggregate blocks (2p, 2p+1); returns per-block results."""
                heads, Fo, ncols, el, bias, relu = LCFG[layer]
                table = tables[layer]
                shard = shards[layer]
                views = [table[0:LO_LIM, :], table[HI_BASE:HI_BASE + 32768, :]]
                gA, gB = geo[p][0], geo[p][1]
                tkL, tkH = gA["tk"], gB["tk"]
                ntt = tkL + tkH
                ba = PAIR_BLOCKS * p
                blocks = [(0, ba)] + ([(1, ba + 1)]
                                      if PAIR_BLOCKS == 2 and ba + 1 < NB
                                      else [])

                # paired gathers (critical Q7 stream)
                gtiles = []
                col0 = idx_off[p]
                for kind in range(2):
                    gk = geo[p][kind]
                    kep = gk["tk"] * 128
                    g = gpool.tile([128, gk["tk"], el], bf16, tag="g")
                    nc.gpsimd.dma_gather(
                        g[:], views[kind],
                        idx16_s[:, col0:col0 + kep // 16],
                        kep, gk["skp"], el,
                        single_packet=SINGLE_PACKET)
                    col0 += kep // 16
                    gtiles.append(g)

                # per-block one-hot M/MT + adp into the shared pair pad_
                pad_ = pppad.tile([128, ntt * heads], f32, tag="adp",
                                  space="PSUM")
                Ms = {}

                def tile_writers(gt):
                    """Blocks covering pair-tile gt (for adp start/stop)."""
                    kind = 0 if gt < tkL else 1
                    t = gt - (0 if kind == 0 else tkL)
                    gk = geo[p][kind]
                    ws = []
                    for x, blk in blocks:
                        rng = gk["a"] if x == 0 else gk["b"]
                        if rng[0] <= t < rng[1]:
                            ws.append(x)
                    return ws

                for x, blk in blocks:
                    nt_lo, nt_hi = blk_nt[blk]
                    ntb = nt_lo + nt_hi
                    off = blk_off[blk]
                    M = mpool.tile([128, max_nt, 128], bf16, tag="M")
                    nc.vector.tensor_tensor(
                        out=M[:, :ntb, :],
                        in0=dstc_s[:, off:off + ntb].unsqueeze(-1)
                            .broadcast_to([128, ntb, 128]),
                        in1=iota_row_s[:].unsqueeze(1)
                            .broadcast_to([128, ntb, 128]),
                        op=Alu.is_equal)
                    Ms[x] = (M, ntb)
                    MT = mpool.tile([128, max_nt * 128], bf16, tag="MT")
                    dr = spool.tile([1, max_nt * 128], bf16, tag="dr")
                    nc.sync.dma_start(out=dr[:, :ntb * 128],
                                      in_=dstr_d[blk:blk + 1, :ntb * 128])
                    for o in range(0, ntb * 128, 512):
                        wd = min(512, ntb * 128 - o)
                        pr = pprep.tile([128, 512], f32, tag="rep",
                                        space="PSUM")
                        nc.tensor.matmul(out=pr[:, :wd], lhsT=ones1_s[:],
                                         rhs=dr[:, o:o + wd],
                                         start=True, stop=True)
                        nc.vector.tensor_tensor(
                            out=MT[:, o:o + wd], in0=pr[:, :wd],
                            in1=iota_col_s[:].broadcast_to([128, wd]),
                            op=Alu.is_equal)
                    # adp matmuls over this block's tiles
                    for kind in range(2):
                        gk = geo[p][kind]
                        t0, t1 = gk["a"] if x == 0 else gk["b"]
                        jbase = 0 if kind == 0 else nt_lo
                        gtb = 0 if kind == 0 else tkL
                        for t in range(t0, t1):
                            gt = gtb + t
                            ws = tile_writers(gt)
                            nc.tensor.matmul(
                                out=pad_[:, gt * heads:(gt + 1) * heads],
                                lhsT=MT[:, (jbase + t - t0) * 128:
                                        (jbase + t - t0 + 1) * 128],
                                rhs=ad_all[layer][:,
                                                  blk * heads:
                                                  (blk + 1) * heads],
                                start=(ws[0] == x), stop=(ws[-1] == x))

                # z for all pair tiles + self-z tails (one group per block)
                nzc = ntt * heads
                nself = len(blocks) * heads
                z = spool.tile([128, nzc + nself], f32, tag="z")
                for kind in range(2):
                    gk = geo[p][kind]
                    zof = (0 if kind == 0 else tkL) * heads
                    nc.vector.tensor_tensor(
                        out=z[:, zof:zof + gk["tk"] * heads]
                            .rearrange("p (t h) -> p t h", t=gk["tk"]),
                        in0=gtiles[kind][:, :, Fo:Fo + heads],
                        in1=pad_[:, zof:zof + gk["tk"] * heads]
                            .rearrange("p (t h) -> p t h", t=gk["tk"]),
                        op=Alu.add)
                for x, blk in blocks:
                    zof = nzc + x * heads
                    nc.vector.tensor_tensor(
                        out=z[:, zof:zof + heads],
                        in0=as_all[layer][:, blk * heads:(blk + 1) * heads],
                        in1=ad_all[layer][:, blk * heads:(blk + 1) * heads],
                        op=Alu.add)
                zl = spool.tile([128, nzc + nself], f32, tag="zl")
                nc.vector.scalar_tensor_tensor(
                    out=zl[:], in0=z[:], scalar=0.2, in1=z[:],
                    op0=Alu.mult, op1=Alu.max)
                tmpS = wpool.tile([128, ntt, Fo + heads], bf16, tag="tmpS")
                nc.scalar.activation(
                    tmpS[:, :, Fo:Fo + heads],
                    zl[:, :nzc].rearrange("p (t h) -> p t h", t=ntt),
                    Act.Exp)
                ses = spool.tile([128, nself], f32, tag="ses")
                nc.scalar.activation(ses[:], zl[:, nzc:nzc + nself], Act.Exp)
                sv = tmpS[:, :, Fo:Fo + heads]
                for kind in range(2):
                    gk = geo[p][kind]
                    ts0 = 0 if kind == 0 else tkL
                    for hh in range(heads):
                        nc.vector.tensor_tensor(
                            out=tmpS[:, ts0:ts0 + gk["tk"],
                                     hh * HID:(hh + 1) * HID],
                            in0=gtiles[kind][:, :, hh * HID:(hh + 1) * HID],
                            in1=sv[:, ts0:ts0 + gk["tk"], hh:hh + 1]
                                .broadcast_to([128, gk["tk"], HID]),
                            op=Alu.mult)

                # per-block fused (agg | den) + epilogue
                results = []
                for x, blk in blocks:
                    M, ntb = Ms[x]
                    nt_lo, _nt_hi = blk_nt[blk]
                    pagg = ppagg.tile([128, Fo + heads], f32, tag="agg",
                                      space="PSUM")
                    mms = []
                    for kind in range(2):
                        gk = geo[p][kind]
                        t0, t1 = gk["a"] if x == 0 else gk["b"]
                        jbase = 0 if kind == 0 else nt_lo
                        gtb = 0 if kind == 0 else tkL
                        for t in range(t0, t1):
                            mms.append((jbase + t - t0, gtb + t))
                    for i, (j, gt) in enumerate(mms):
                        nc.tensor.matmul(
                            out=pagg[:],
                            lhsT=M[:, j, :],
                            rhs=tmpS[:, gt, :],
                            start=(i == 0), stop=(i == len(mms) - 1))

                    se = spool.tile([128, heads], bf16, tag="se")
                    nc.vector.tensor_tensor(
                        out=se[:], in0=ses[:, x * heads:(x + 1) * heads],
                        in1=mselfc_s[:, blk:blk + 1]
                            .broadcast_to([128, heads]),
                        op=Alu.mult)
                    h_own = wpool.tile([128, Fo], bf16, tag="hown")
                    nc.sync.dma_start(out=h_own[:],
                                      in_=shard[blk * BS:(blk + 1) * BS, :Fo])
                    hs = wpool.tile([128, Fo + heads], bf16, tag="hs")
                    for hh in range(heads):
                        nc.vector.tensor_tensor(
                            out=hs[:, hh * HID:(hh + 1) * HID],
                            in0=h_own[:, hh * HID:(hh + 1) * HID],
                            in1=se[:, hh:hh + 1].broadcast_to([128, HID]),
                            op=Alu.mult)
                    nc.vector.tensor_copy(out=hs[:, Fo:Fo + heads], in_=se[:])

                    t1_ = wpool.tile([128, Fo + heads], f32, tag="t1")
                    nc.vector.tensor_tensor(out=t1_[:], in0=pagg[:],
                                            in1=hs[:], op=Alu.add)
                    den = spool.tile([128, heads], f32, tag="den")
                    nc.vector.tensor_scalar(out=den[:],
                                            in0=t1_[:, Fo:Fo + heads],
                                            scalar1=1e-16, scalar2=None,
                                            op0=Alu.add)
                    rec = spool.tile([128, heads], f32, tag="rec")
                    nc.vector.reciprocal(out=rec[:], in_=den[:])
                    o1 = wpool.tile([128, Fo], f32, tag="o1")
                    nc.vector.tensor_tensor(
                        out=o1[:].rearrange("p (h f) -> p h f", h=heads),
                        in0=t1_[:, :Fo].rearrange("p (h f) -> p h f",
                                                  h=heads),
                        in1=rec[:].unsqueeze(-1)
                            .broadcast_to([128, heads, HID]),
                        op=Alu.mult)
                    o2 = wpool.tile([128, Fo], bf16, tag="o2")
                    nc.vector.tensor_tensor(out=o2[:], in0=o1[:],
                                            in1=bias[:, :Fo], op=Alu.add)
                    if layer == 2:
                        results.append((blk, o2))
                        continue
                    o3 = wpool.tile([128, Fo], bf16, tag="o3")
                    nc.vector.tensor_scalar(out=o3[:], in0=o2[:],
                                            scalar1=0.0, scalar2=None,
                                            op0=Alu.max)
                    hTb = []
                    for k2 in range(2):
                        pt = pptp.tile([128, 128], bf16, tag="tp",
                                       space="PSUM")
                        nc.tensor.transpose(pt[:],
                                            o3[:, k2 * 128:(k2 + 1) * 128],
                                            ident_s[:])
                        ht = spool.tile([128, 128], bf16, tag=f"ht{k2}")
                        nc.vector.tensor_copy(out=ht[:], in_=pt[:])
                        hTb.append(ht)
                    results.append((blk, hTb))
                return results

            # ---- layer 0 transform (batched x^T loads) ----
            for b0_ in range(0, NB, 4):
                nbk = min(4, NB - b0_)
                xb = wpool.tile([IN_C, 4 * BS], bf16, tag="xtb")
                nc.sync.dma_start(
                    out=xb[:, :nbk * BS].rearrange("c (b s) -> c b s", b=nbk),
                    in_=xtb_d[b0_:b0_ + nbk].rearrange("b c s -> c b s"))
                for j in range(nbk):
                    transform_block(0, b0_ + j,
                                    xb[:, j * BS:(j + 1) * BS], None)
            allgather(0)

            # prime gather tiles (stale-tail rows must be finite)
            for _ in range(GBUFS):
                g = gpool.tile([128, max_tk, EL01], bf16, tag="g")
                nc.vector.memset(g[:], 0.0)

            # ---- layer 0/1: aggregate + interleaved next transform ----
            for layer in range(2):
                for p in range(NPAIR):
                    for blk, hTb in agg_pair(layer, p):
                        transform_block(layer + 1, blk, hTb[0][:], hTb[1][:])
                allgather(layer + 1)

            # ---- layer 2: aggregate + masked column sum ----
            psum_sum = ppsum.tile([1, OUT_C], f32, tag="sum", space="PSUM")
            first = True
            for p in range(NPAIR):
                for blk, o2 in agg_pair(2, p):
                    nc.tensor.matmul(out=psum_sum[:],
                                     lhsT=maskc_s[:, blk:blk + 1],
                                     rhs=o2[:], start=first,
                                     stop=(blk == NB - 1))
                    first = False
            osb = spool.tile([1, OUT_C], f32, tag="osb")
            nc.vector.tensor_copy(out=osb[:], in_=psum_sum[:])
            nc.sync.dma_start(out=out_d[:], in_=osb[:])

    nc.compile()
    return nc


def _geo_key(pp):
    return repr([(g[0]["sa"], g[0]["sb"], g[1]["sa"], g[1]["sb"])
                 for g in pp["geo"]])


def _get_built(pp):
    global _BUILT, _BUILT_KEY
    key = _geo_key(pp)
    if _BUILT is None or _BUILT_KEY != key:
        _BUILT = build_kernel(pp["geo"], pp["idx_off"], pp["blk_off"],
                              pp["blk_nt"], pp["idx16"].shape[2],
                              pp["dstc"].shape[2], pp["dstr"].shape[2])
        _BUILT_KEY = key
    return _BUILT


def kernel(**inputs) -> np.ndarray:
    from concourse.bass_utils import run_bass_kernel_spmd

    pp = preprocess(np.asarray(inputs["edge_index"]))
    in_maps = build_core_inputs(inputs, pp)
    nc = _get_built(pp)
    res = run_bass_kernel_spmd(nc, in_maps, core_ids=list(range(NCORES)))
    parts = np.stack([r["out_part"][0] for r in res.results])  # [8, 64]
    g = parts.sum(axis=0, keepdims=True) / N
    out = (g @ np.asarray(inputs["hw"], np.float32)
           + np.asarray(inputs["hb"], np.float32)).astype(np.float32)
    return out


# revision 24
# speedup vs baseline: 1.2062x; 1.2062x over previous
"""3-layer GAT on 8 trn2 NeuronCores.

Strategy (graph/data parallel per sharding hint):
  - Nodes assigned to 8 cores x 49 blocks x 128 slots (degree-balanced LPT
    packing) -> permuted node order; table row = core*6272 + block*128 + slot.
  - Per layer, per node block: transform with rhs = [W | W@as | W@ad] (alpha
    terms folded into the matmul) -> bf16 table shard [6272, 384]; one
    AllGather (Shared output) per layer -> full table on every core.
  - Aggregation processes PAIRS of dst blocks: non-self edges of both blocks
    (dst-sorted) share one dma_gather per int16-index window (lo rows
    [0,32768), hi rows [17408,50176)), amortizing the Q7 descriptor-gen
    fixed cost -- the gather stream is the kernel's critical path.  One-hot
    scatter matrices M [edge,dst] / MT [dst,edge] are built on-device per
    block over its (static) tile range and feed matmuls for the per-edge ad
    term and the fused (feature | denom) accumulation in PSUM.  Self-loop
    contributions are computed from the local shard and never gathered.
  - Softmax max-shift skipped (logits O(1), exp safe; mathematically equal).
  - Next layer's transform is interleaved into the aggregation loop
    (block-level pipeline); layer 2 reduces via a mask matmul; final mean +
    linear head on host.  ACT engine runs only Exp; casts/copies/lrelu/relu
    are DVE ops (ACT copy truncates f32->bf16; DVE rounds).
"""

import os
import numpy as np
import ml_dtypes

# ---------------- problem constants (must match reference) ----------------
N = 50000
E = 800000
IN_C = 128
HID = 64
HEADS = 4
OUT_C = 64
F1 = HEADS * HID  # 256

# ---------------- sharding geometry ----------------
NCORES = 8
NB = 49            # dst blocks per core
BS = 128           # dst slots per block
NPC = NB * BS      # 6272 nodes per core
RTOT = NCORES * NPC  # 50176 table rows
KE_CAP = 1152      # lo/hi packing capacity per (block, kind)
LO_LIM = 32768     # lo window rows [0, 32768)
HI_BASE = 17408    # hi window rows [17408, 50176)
# Aggregation unit: PAIR_BLOCKS=2 shares one dma_gather per window between
# two dst blocks (fewer Q7 fixed costs, but bigger gathers stall the SWDGE
# ring); PAIR_BLOCKS=1 gathers per block (measured faster on HW).
PAIR_BLOCKS = int(os.environ.get("GAT_PAIR", "1"))
NPAIR = (NB + PAIR_BLOCKS - 1) // PAIR_BLOCKS

EL01 = 384         # table elems/row layers 0/1: 256 h + 4 as + 4 ad + pad
EL2 = 128          # table elems/row layer 2: 64 h + 1 as + 1 ad + pad
BF = ml_dtypes.bfloat16

GBUFS = 6          # gather tile double-buffer depth
SINGLE_PACKET = os.environ.get("GAT_SP", "0") == "1"


def _a16(x):
    return (int(x) + 15) // 16 * 16


# ---------------- host preprocessing ----------------

def preprocess(edge_index):
    """Node->(core,block,slot) assignment, pair-packed edge arrays, and the
    static pair geometry consumed by build_kernel."""
    import heapq

    e0 = np.asarray(edge_index[0], np.int64)
    e1 = np.asarray(edge_index[1], np.int64)
    nonself = e0 != e1
    src = e0[nonself]
    dst = e1[nonself]
    # self-edge multiplicity: 1 (PyG added loop) + natural self edges
    mult = np.ones(N, np.int64)
    np.add.at(mult, e0[~nonself], 1)

    deg = np.bincount(dst, minlength=N)  # gather load per dst node

    nblocks = NCORES * NB
    order = np.argsort(-deg, kind="stable")
    heap = [(0, b) for b in range(nblocks)]
    heapq.heapify(heap)
    slots_used = np.zeros(nblocks, np.int64)
    node_block = np.empty(N, np.int64)
    node_slot = np.empty(N, np.int64)
    for n in order:
        while True:
            load, b = heapq.heappop(heap)
            if slots_used[b] < BS:
                break
        node_block[n] = b
        node_slot[n] = slots_used[b]
        slots_used[b] += 1
        heapq.heappush(heap, (load + int(deg[n]), b))

    row = node_block * BS + node_slot  # block-major global table row

    xperm = np.full(RTOT, -1, np.int64)
    xperm[row] = np.arange(N)

    erow = row[src]
    eblk = node_block[dst]
    eslot = node_slot[dst]

    order_e = np.argsort(eblk, kind="stable")
    bounds = np.searchsorted(eblk[order_e], np.arange(nblocks + 1))

    # pass 1: split lo/hi per block, count
    packed = {}
    cnt = np.zeros((NCORES, NB, 2), np.int64)
    for b in range(nblocks):
        c, bl = divmod(b, NB)
        es = order_e[bounds[b]:bounds[b + 1]]
        r_ = erow[es]
        dl = eslot[es]
        lo_f = r_ < HI_BASE
        hi_f = r_ >= LO_LIM
        flex = ~lo_f & ~hi_f
        n_lo = int(lo_f.sum())
        n_hi = int(hi_f.sum())
        n_fx = int(flex.sum())
        tot = n_lo + n_hi + n_fx
        assert tot <= 2 * KE_CAP, f"block {b} has {tot} edges > {2*KE_CAP}"
        add_lo = min(n_fx, max(0, min(KE_CAP, (tot + 1) // 2) - n_lo))
        if n_hi + (n_fx - add_lo) > KE_CAP:
            add_lo = n_fx - (KE_CAP - n_hi)
        assert 0 <= add_lo <= n_fx
        fx_idx = np.nonzero(flex)[0]
        sel_lo = np.zeros(len(es), bool)
        sel_lo[lo_f] = True
        sel_lo[fx_idx[:add_lo]] = True
        for kind, sel, base in ((0, sel_lo, 0), (1, ~sel_lo, HI_BASE)):
            rr = r_[sel]
            dd = dl[sel]
            o = np.argsort(rr, kind="stable")  # DMA locality
            packed[(b, kind)] = (rr[o] - base, dd[o])
            cnt[c, bl, kind] = len(rr)

    # ---- static pair geometry ----
    # pair p = blocks (2p, 2p+1); per (pair, kind): section sizes sa/sb
    # (16-aligned max over cores), gather reg = sa+sb, tiles tk.
    # Per block and kind: tile range [t0, t1) within the pair's tiles.
    geo = []
    for p in range(NPAIR):
        ba = PAIR_BLOCKS * p
        bb = ba + 1
        has_b = PAIR_BLOCKS == 2 and bb < NB
        pk = []
        for kind in range(2):
            sa = _a16(cnt[:, ba, kind].max())
            sb = _a16(cnt[:, bb, kind].max()) if has_b else 0
            skp = sa + sb
            tk = (skp + 127) // 128
            a0, a1 = 0, (sa + 127) // 128
            b0, b1 = sa // 128, tk
            pk.append(dict(sa=sa, sb=sb, skp=skp, tk=tk,
                           a=(a0, a1), b=(b0, b1) if has_b else None))
        geo.append(pk)

    # flat column offsets for idx16 / per-block dstc (dstcX: per block, both
    # kinds adjacent: [lo tiles | hi tiles] over that block's tile ranges)
    idx_off = []
    o = 0
    for p in range(NPAIR):
        idx_off.append(o)
        o += (geo[p][0]["tk"] * 128 + geo[p][1]["tk"] * 128) // 16
    idx_cols = o

    blk_off = []   # per block: column offset into dstc flat (tile units)
    blk_nt = []    # per block: (nt_lo, nt_hi)
    o = 0
    for bl in range(NB):
        p, x = divmod(bl, PAIR_BLOCKS)
        rng = [geo[p][k]["a" if x == 0 else "b"] for k in range(2)]
        nt = [r[1] - r[0] for r in rng]
        blk_off.append(o)
        blk_nt.append(tuple(nt))
        o += nt[0] + nt[1]
    dstc_cols = o

    dstr_cols = max((blk_nt[b][0] + blk_nt[b][1]) * 128 for b in range(NB))
    idx16 = np.full((NCORES, 128, idx_cols), -1, np.int16)
    dstc = np.full((NCORES, 128, dstc_cols), -1.0, np.float32)
    dstr = np.full((NCORES, NB, dstr_cols), -1.0, np.float32)
    maskc = np.zeros((NCORES, 128, NB), np.float32)
    mselfc = np.zeros((NCORES, 128, NB), np.float32)

    for c in range(NCORES):
        for p in range(NPAIR):
            ba = PAIR_BLOCKS * p
            bb = ba + 1
            has_b = PAIR_BLOCKS == 2 and bb < NB
            col0 = idx_off[p]
            for kind in range(2):
                gk = geo[p][kind]
                sa, sb, skp, tk = gk["sa"], gk["sb"], gk["skp"], gk["tk"]
                kep = tk * 128
                relA, ddA = packed[(c * NB + ba, kind)]
                full = np.full(kep, -1, np.int64)
                dloc = np.full(kep, -1.0, np.float32)
                kA = len(relA)
                full[:kA] = relA
                full[kA:sa] = 0
                dloc[:kA] = ddA.astype(np.float32)
                if has_b:
                    relB, ddB = packed[(c * NB + bb, kind)]
                    kB = len(relB)
                    full[sa:sa + kB] = relB
                    full[sa + kB:skp] = 0
                    dloc[sa:sa + kB] = ddB.astype(np.float32)
                w = full.reshape(kep // 16, 16).T.astype(np.int16)
                idx16[c, :, col0:col0 + kep // 16] = np.tile(w, (8, 1))
                col0 += kep // 16
                # per-block dstc over tile ranges (other block's slots = -1)
                for x, blk in ((0, ba),) + (((1, bb),) if has_b else ()):
                    t0, t1 = gk["a"] if x == 0 else gk["b"]
                    dl = dloc.copy()
                    if x == 0:
                        dl[sa:] = -1.0
                    else:
                        dl[:sa] = -1.0
                    seg = dl.reshape(tk, 128)[t0:t1]          # [nt, 128]
                    off = blk_off[blk] + (0 if kind == 0 else blk_nt[blk][0])
                    nt = t1 - t0
                    dstc[c, :, off:off + nt] = seg.T
                    dstr[c, blk, (0 if kind == 0 else
                                  blk_nt[blk][0] * 128):][:nt * 128] = \
                        seg.reshape(-1)

        for bl in range(NB):
            b = c * NB + bl
            used = slots_used[b]
            maskc[c, :used, bl] = 1.0
            nodes_b = np.where(node_block == b)[0]
            mselfc[c, node_slot[nodes_b], bl] = \
                mult[nodes_b].astype(np.float32)

    return dict(row=row, xperm=xperm, idx16=idx16, dstc=dstc, dstr=dstr,
                maskc=maskc, mselfc=mselfc, cnt=cnt, geo=geo,
                idx_off=idx_off, blk_off=blk_off, blk_nt=blk_nt,
                node_block=node_block, node_slot=node_slot)


def host_weights(inputs):
    """Extended weight matrices with folded attention vectors."""
    def ext(W, a_s, a_d, heads):
        Wh = W.reshape(W.shape[0], heads, HID)
        Was = np.einsum("khc,hc->kh", Wh, a_s)
        Wad = np.einsum("khc,hc->kh", Wh, a_d)
        return np.concatenate([W, Was, Wad], axis=1).astype(np.float32)

    W0e = ext(np.asarray(inputs["W0"], np.float32),
              np.asarray(inputs["a0s"], np.float32),
              np.asarray(inputs["a0d"], np.float32), HEADS)      # [128, 264]
    W1e = ext(np.asarray(inputs["W1"], np.float32),
              np.asarray(inputs["a1s"], np.float32),
              np.asarray(inputs["a1d"], np.float32), HEADS)      # [256, 264]
    W2e = ext(np.asarray(inputs["W2"], np.float32),
              np.asarray(inputs["a2s"], np.float32),
              np.asarray(inputs["a2d"], np.float32), 1)          # [256, 66]
    return W0e, W1e, W2e


def build_core_inputs(inputs, pp):
    """Per-core in_maps for run_bass_kernel_spmd."""
    x = np.asarray(inputs["x"], np.float32)
    W0e, W1e, W2e = host_weights(inputs)
    b0 = np.asarray(inputs["b0"], np.float32)
    b1 = np.asarray(inputs["b1"], np.float32)
    b2 = np.asarray(inputs["b2"], np.float32)

    iota_row = np.tile(np.arange(128, dtype=np.float32), (128, 1))
    iota_col = np.arange(128, dtype=np.float32).reshape(128, 1)
    ones1 = np.ones((1, 128), np.float32)
    ident = np.eye(128, dtype=np.float32)

    consts = dict(
        w0e=W0e.astype(BF),
        w1e=W1e.reshape(2, 128, F1 + 2 * HEADS).astype(BF),
        w2e=W2e.reshape(2, 128, HID + 2).astype(BF),
        b0r=np.tile(b0, (128, 1)).astype(BF),
        b1r=np.tile(b1, (128, 1)).astype(BF),
        b2r=np.tile(b2, (128, 1)).astype(BF),
        iota_row=iota_row.astype(BF), iota_col=iota_col.astype(BF),
        ones1=ones1.astype(BF), ident=ident.astype(BF),
    )

    in_maps = []
    for c in range(NCORES):
        xtb = np.zeros((NB, IN_C, BS), np.float32)
        rows = np.arange(c * NPC, (c + 1) * NPC)
        nodes = pp["xperm"][rows].reshape(NB, BS)
        for b in range(NB):
            nb = nodes[b]
            valid = nb >= 0
            if valid.any():
                xtb[b][:, valid] = x[nb[valid]].T
        m = dict(
            xtb=xtb.astype(BF),
            idx16=pp["idx16"][c],
            dstc=pp["dstc"][c].astype(BF),
            dstr=pp["dstr"][c].astype(BF),
            maskc=pp["maskc"][c].astype(BF),
            mselfc=pp["mselfc"][c].astype(BF),
            **consts,
        )
        in_maps.append(m)
    return in_maps


# ---------------- numpy emulation of the device data path ----------------

def _emulate_layer(tables_in, pp, We, bias, heads, Fo, relu, el):
    """tables_in: node-major feature mat [RTOT, F_in] (f32).
    Mirrors the pair-packed device data path (reads pp's flat arrays)."""
    geo = pp["geo"]
    ncols = Fo + 2 * heads
    tb = (tables_in.astype(BF).astype(np.float32)
          @ We.astype(BF).astype(np.float32))
    table = np.zeros((RTOT, el), BF)
    table[:, :ncols] = tb.astype(BF)
    as_all = tb[:, Fo:Fo + heads].astype(BF).astype(np.float32)
    ad_all = tb[:, Fo + heads:Fo + 2 * heads].astype(BF).astype(np.float32)

    def lrexp(z):
        return np.exp(np.maximum(z, 0.2 * z)).astype(np.float32)

    out = np.zeros((RTOT, Fo), np.float32)
    for c in range(NCORES):
        for p in range(len(geo)):
            col0 = pp["idx_off"][p]
            gs = []
            dls = []
            for kind in range(2):
                gk = geo[p][kind]
                kep = gk["tk"] * 128
                w = pp["idx16"][c][:16, col0:col0 + kep // 16]
                col0 += kep // 16
                rel = w.T.reshape(-1).astype(np.int64)
                r = gk["skp"]
                base = 0 if kind == 0 else HI_BASE
                rows = rel[:r] + base
                g = np.zeros((kep, el), np.float32)
                g[:r] = np.asarray(table[rows], np.float32)
                gs.append(g)
            ba = PAIR_BLOCKS * p
            blks = ((0, ba),) + (((1, ba + 1),)
                                 if PAIR_BLOCKS == 2 and ba + 1 < NB else ())
            for x, blk in blks:
                bl = blk
                rbase = c * NPC + bl * BS
                agg = np.zeros((BS, Fo), np.float32)
                den = np.zeros((BS, heads), np.float32)
                for kind in range(2):
                    gk = geo[p][kind]
                    t0, t1 = gk["a"] if x == 0 else gk["b"]
                    off = pp["blk_off"][bl] + (
                        0 if kind == 0 else pp["blk_nt"][bl][0])
                    nt = t1 - t0
                    dl = pp["dstc"][c][:, off:off + nt].T.reshape(-1)
                    dl = dl.astype(np.int64)                    # [nt*128]
                    g = gs[kind][t0 * 128:t1 * 128]             # [nt*128, el]
                    valid = dl >= 0
                    a_s = g[:, Fo:Fo + heads]
                    a_d = np.where(valid[:, None],
                                   ad_all[rbase + dl % 128 if False else
                                          rbase + np.clip(dl, 0, None)], 0.0)
                    s = lrexp(a_s + a_d).astype(BF).astype(np.float32)
                    s = np.where(valid[:, None], s, 0.0)
                    hsc = (g[:, :Fo].reshape(-1, heads, HID)
                           * s[:, :, None]).astype(BF).astype(np.float32)
                    hsc = hsc.reshape(-1, Fo)
                    np.add.at(agg, np.clip(dl, 0, None)[valid], hsc[valid])
                    np.add.at(den, np.clip(dl, 0, None)[valid], s[valid])
                # self loops
                ms = pp["mselfc"][c][:, bl]
                ss = lrexp(as_all[rbase:rbase + BS]
                           + ad_all[rbase:rbase + BS])
                se = (ss * ms[:, None]).astype(BF).astype(np.float32)
                h_own = np.asarray(table[rbase:rbase + BS, :Fo], np.float32)
                hs = (h_own.reshape(BS, heads, HID)
                      * se[:, :, None]).astype(BF).astype(np.float32)
                agg += hs.reshape(BS, Fo)
                den += se
                o = agg.reshape(BS, heads, HID) / (den + 1e-16)[:, :, None]
                o = o.reshape(BS, Fo) + bias
                if relu:
                    o = np.maximum(o, 0.0)
                out[rbase:rbase + BS] = o
    return out


def emulate(inputs, pp=None):
    """Full numpy emulation; returns [1, OUT_C]."""
    if pp is None:
        pp = preprocess(np.asarray(inputs["edge_index"]))
    x = np.asarray(inputs["x"], np.float32)
    W0e, W1e, W2e = host_weights(inputs)
    h = np.zeros((RTOT, IN_C), np.float32)
    valid = pp["xperm"] >= 0
    h[valid] = x[pp["xperm"][valid]]

    b0 = np.asarray(inputs["b0"], np.float32)
    b1 = np.asarray(inputs["b1"], np.float32)
    b2 = np.asarray(inputs["b2"], np.float32)

    h0 = _emulate_layer(h, pp, W0e, b0, HEADS, F1, True, EL01)
    h1 = _emulate_layer(h0, pp, W1e, b1, HEADS, F1, True, EL01)
    h2 = _emulate_layer(h1, pp, W2e, b2, 1, HID, False, EL2)

    g = h2[valid].sum(axis=0, keepdims=True) / N
    return (g @ np.asarray(inputs["hw"], np.float32)
            + np.asarray(inputs["hb"], np.float32)).astype(np.float32)


# ---------------- device kernel ----------------

_BUILT = None
_BUILT_KEY = None


def build_kernel(geo, idx_off, blk_off, blk_nt, idx_cols, dstc_cols,
                 dstr_cols):
    import concourse.bacc as bacc
    import concourse.mybir as mybir
    import concourse.tile as tile
    from concourse import library_config

    f32 = mybir.dt.float32
    bf16 = mybir.dt.bfloat16
    i16 = mybir.dt.int16
    Alu = mybir.AluOpType
    Act = mybir.ActivationFunctionType

    nc = bacc.Bacc("TRN2", target_bir_lowering=False, debug=False,
                   num_devices=NCORES, num_swdge_queues=2)

    max_tk = max(g[k]["tk"] for g in geo for k in range(2))
    max_nt = max(nt[0] + nt[1] for nt in blk_nt)

    # ---- I/O ----
    xtb_d = nc.dram_tensor("xtb", [NB, IN_C, BS], bf16, kind="ExternalInput")
    idx16_d = nc.dram_tensor("idx16", [128, idx_cols], i16,
                             kind="ExternalInput")
    dstc_d = nc.dram_tensor("dstc", [128, dstc_cols], bf16,
                            kind="ExternalInput")
    dstr_d = nc.dram_tensor("dstr", [NB, dstr_cols], bf16,
                            kind="ExternalInput")
    maskc_d = nc.dram_tensor("maskc", [128, NB], bf16, kind="ExternalInput")
    mselfc_d = nc.dram_tensor("mselfc", [128, NB], bf16, kind="ExternalInput")
    w0e_d = nc.dram_tensor("w0e", [IN_C, F1 + 2 * HEADS], bf16,
                           kind="ExternalInput")
    w1e_d = nc.dram_tensor("w1e", [2, 128, F1 + 2 * HEADS], bf16,
                           kind="ExternalInput")
    w2e_d = nc.dram_tensor("w2e", [2, 128, HID + 2], bf16,
                           kind="ExternalInput")
    b0r_d = nc.dram_tensor("b0r", [128, F1], bf16, kind="ExternalInput")
    b1r_d = nc.dram_tensor("b1r", [128, F1], bf16, kind="ExternalInput")
    b2r_d = nc.dram_tensor("b2r", [128, HID], bf16, kind="ExternalInput")
    iota_row_d = nc.dram_tensor("iota_row", [128, 128], bf16,
                                kind="ExternalInput")
    iota_col_d = nc.dram_tensor("iota_col", [128, 1], bf16,
                                kind="ExternalInput")
    ones1_d = nc.dram_tensor("ones1", [1, 128], bf16, kind="ExternalInput")
    ident_d = nc.dram_tensor("ident", [128, 128], bf16, kind="ExternalInput")
    out_d = nc.dram_tensor("out_part", [1, OUT_C], f32, kind="ExternalOutput")

    # internal DRAM
    tables = []
    shards = []
    for li, el in enumerate([EL01, EL01, EL2]):
        tables.append(nc.dram_tensor(f"table{li}", [RTOT, el], bf16,
                                     addr_space="Shared"))
        shards.append(nc.dram_tensor(f"shard{li}", [NPC, el], bf16))

    rg = [list(range(NCORES))]

    with tile.TileContext(nc) as tc:
        with (
            tc.tile_pool(name="const", bufs=1) as cpool,
            tc.tile_pool(name="gather", bufs=GBUFS) as gpool,
            tc.tile_pool(name="onehot", bufs=4) as mpool,
            tc.tile_pool(name="work", bufs=3) as wpool,
            tc.tile_pool(name="small", bufs=4) as spool,
            tc.tile_pool(name="adas", bufs=1) as apool,
            tc.tile_pool(name="ps_agg", bufs=2, space="PSUM") as ppagg,
            tc.tile_pool(name="ps_pad", bufs=2, space="PSUM") as pppad,
            tc.tile_pool(name="ps_rep", bufs=1, space="PSUM") as pprep,
            tc.tile_pool(name="ps_tp", bufs=1, space="PSUM") as pptp,
            tc.tile_pool(name="ps_tf", bufs=1, space="PSUM") as pptf,
            tc.tile_pool(name="ps_sum", bufs=1, space="PSUM") as ppsum,
        ):
            def load_const(tag, dram, shape, dtype=bf16, view=None):
                t = cpool.tile(shape, dtype, tag=tag)
                nc.sync.dma_start(out=t[:], in_=view if view is not None
                                  else dram[:])
                return t

            w0e_s = load_const("w0e", w0e_d, [IN_C, F1 + 2 * HEADS])
            w1e_s = load_const("w1e", w1e_d, [128, 2, F1 + 2 * HEADS],
                               view=w1e_d[:].rearrange("c p j -> p c j"))
            w2e_s = load_const("w2e", w2e_d, [128, 2, HID + 2],
                               view=w2e_d[:].rearrange("c p j -> p c j"))
            b0r_s = load_const("b0r", b0r_d, [128, F1])
            b1r_s = load_const("b1r", b1r_d, [128, F1])
            b2r_s = load_const("b2r", b2r_d, [128, HID])
            iota_row_s = load_const("iota_row", iota_row_d, [128, 128])
            iota_col_s = load_const("iota_col", iota_col_d, [128, 1])
            ones1_s = load_const("ones1", ones1_d, [1, 128])
            ident_s = load_const("ident", ident_d, [128, 128])
            idx16_s = load_const("idx16", idx16_d, [128, idx_cols], i16)
            dstc_s = load_const("dstc", dstc_d, [128, dstc_cols])
            maskc_s = load_const("maskc", maskc_d, [128, NB])
            mselfc_s = load_const("mselfc", mselfc_d, [128, NB])

            nc.gpsimd.load_library(library_config.mlp)

            # persistent per-layer alpha tiles [128, NB*heads]
            as_all0 = apool.tile([128, NB * HEADS], bf16, tag="as0")
            as_all1 = apool.tile([128, NB * HEADS], bf16, tag="as1")
            as_all2 = apool.tile([128, NB], bf16, tag="as2")
            ad_all0 = apool.tile([128, NB * HEADS], bf16, tag="ad0")
            ad_all1 = apool.tile([128, NB * HEADS], bf16, tag="ad1")
            ad_all2 = apool.tile([128, NB], bf16, tag="ad2")
            as_all = [as_all0, as_all1, as_all2]
            ad_all = [ad_all0, ad_all1, ad_all2]

            LCFG = [  # heads, Fo, ncols, el, bias, relu
                (HEADS, F1, F1 + 2 * HEADS, EL01, b0r_s, True),
                (HEADS, F1, F1 + 2 * HEADS, EL01, b1r_s, True),
                (1, HID, HID + 2, EL2, b2r_s, False),
            ]

            def transform_block(layer, b, lhsT0, lhsT1):
                heads, Fo, ncols, el, _bias, _relu = LCFG[layer]
                shard = shards[layer]
                ps = pptf.tile([128, 512], f32, tag="tf", space="PSUM")
                if layer == 0:
                    nc.tensor.matmul(out=ps[:, :ncols], lhsT=lhsT0,
                                     rhs=w0e_s[:], start=True, stop=True)
                else:
                    we = w1e_s if layer == 1 else w2e_s
                    nc.tensor.matmul(out=ps[:, :ncols], lhsT=lhsT0,
                                     rhs=we[:, 0, :], start=True, stop=False)
                    nc.tensor.matmul(out=ps[:, :ncols], lhsT=lhsT1,
                                     rhs=we[:, 1, :], start=False, stop=True)
                tb = wpool.tile([128, EL01], bf16, tag="tb")
                nc.vector.tensor_copy(out=tb[:, :ncols], in_=ps[:, :ncols])
                nc.vector.tensor_copy(
                    out=as_all[layer][:, b * heads:(b + 1) * heads],
                    in_=ps[:, Fo:Fo + heads])
                nc.vector.tensor_copy(
                    out=ad_all[layer][:, b * heads:(b + 1) * heads],
                    in_=ps[:, Fo + heads:Fo + 2 * heads])
                nc.sync.dma_start(out=shard[b * BS:(b + 1) * BS, :],
                                  in_=tb[:, :el])

            def allgather(layer):
                nc.gpsimd.collective_compute(
                    "AllGather", mybir.AluOpType.bypass,
                    replica_groups=rg, ins=[shards[layer][:].opt()],
                    outs=[tables[layer][:].opt()])

            def agg_pair(layer, p):
                """Aggregate blocks (2p, 2p+1); returns per-block results."""
                heads, Fo, ncols, el, bias, relu = LCFG[layer]
                table = tables[layer]
                shard = shards[layer]
                views = [table[0:LO_LIM, :], table[HI_BASE:HI_BASE + 32768, :]]
                gA, gB = geo[p][0], geo[p][1]
                tkL, tkH = gA["tk"], gB["tk"]
                ntt = tkL + tkH
                ba = PAIR_BLOCKS * p
                blocks = [(0, ba)] + ([(1, ba + 1)]
                                      if PAIR_BLOCKS == 2 and ba + 1 < NB
                                      else [])

                # paired gathers (critical Q7 stream)
                gtiles = []
                col0 = idx_off[p]
                for kind in range(2):
                    gk = geo[p][kind]
                    kep = gk["tk"] * 128
                    g = gpool.tile([128, gk["tk"], el], bf16, tag="g")
                    nc.gpsimd.dma_gather(
                        g[:], views[kind],
                        idx16_s[:, col0:col0 + kep // 16],
                        kep, gk["skp"], el,
                        single_packet=SINGLE_PACKET, queue_num=kind)
                    col0 += kep // 16
                    gtiles.append(g)

                # per-block one-hot M/MT + adp into the shared pair pad_
                pad_ = pppad.tile([128, ntt * heads], f32, tag="adp",
                                  space="PSUM")
                Ms = {}

                def tile_writers(gt):
                    """Blocks covering pair-tile gt (for adp start/stop)."""
                    kind = 0 if gt < tkL else 1
                    t = gt - (0 if kind == 0 else tkL)
                    gk = geo[p][kind]
                    ws = []
                    for x, blk in blocks:
                        rng = gk["a"] if x == 0 else gk["b"]
                        if rng[0] <= t < rng[1]:
                            ws.append(x)
                    return ws

                for x, blk in blocks:
                    nt_lo, nt_hi = blk_nt[blk]
                    ntb = nt_lo + nt_hi
                    off = blk_off[blk]
                    M = mpool.tile([128, max_nt, 128], bf16, tag="M")
                    nc.vector.tensor_tensor(
                        out=M[:, :ntb, :],
                        in0=dstc_s[:, off:off + ntb].unsqueeze(-1)
                            .broadcast_to([128, ntb, 128]),
                        in1=iota_row_s[:].unsqueeze(1)
                            .broadcast_to([128, ntb, 128]),
                        op=Alu.is_equal)
                    Ms[x] = (M, ntb)
                    MT = mpool.tile([128, max_nt * 128], bf16, tag="MT")
                    dr = spool.tile([1, max_nt * 128], bf16, tag="dr")
                    nc.sync.dma_start(out=dr[:, :ntb * 128],
                                      in_=dstr_d[blk:blk + 1, :ntb * 128])
                    for o in range(0, ntb * 128, 512):
                        wd = min(512, ntb * 128 - o)
                        pr = pprep.tile([128, 512], f32, tag="rep",
                                        space="PSUM")
                        nc.tensor.matmul(out=pr[:, :wd], lhsT=ones1_s[:],
                                         rhs=dr[:, o:o + wd],
                                         start=True, stop=True)
                        nc.vector.tensor_tensor(
                            out=MT[:, o:o + wd], in0=pr[:, :wd],
                            in1=iota_col_s[:].broadcast_to([128, wd]),
                            op=Alu.is_equal)
                    # adp matmuls over this block's tiles
                    for kind in range(2):
                        gk = geo[p][kind]
                        t0, t1 = gk["a"] if x == 0 else gk["b"]
                        jbase = 0 if kind == 0 else nt_lo
                        gtb = 0 if kind == 0 else tkL
                        for t in range(t0, t1):
                            gt = gtb + t
                            ws = tile_writers(gt)
                            nc.tensor.matmul(
                                out=pad_[:, gt * heads:(gt + 1) * heads],
                                lhsT=MT[:, (jbase + t - t0) * 128:
                                        (jbase + t - t0 + 1) * 128],
                                rhs=ad_all[layer][:,
                                                  blk * heads:
                                                  (blk + 1) * heads],
                                start=(ws[0] == x), stop=(ws[-1] == x))

                # z for all pair tiles + self-z tails (one group per block)
                nzc = ntt * heads
                nself = len(blocks) * heads
                z = spool.tile([128, nzc + nself], f32, tag="z")
                for kind in range(2):
                    gk = geo[p][kind]
                    zof = (0 if kind == 0 else tkL) * heads
                    nc.vector.tensor_tensor(
                        out=z[:, zof:zof + gk["tk"] * heads]
                            .rearrange("p (t h) -> p t h", t=gk["tk"]),
                        in0=gtiles[kind][:, :, Fo:Fo + heads],
                        in1=pad_[:, zof:zof + gk["tk"] * heads]
                            .rearrange("p (t h) -> p t h", t=gk["tk"]),
                        op=Alu.add)
                for x, blk in blocks:
                    zof = nzc + x * heads
                    nc.vector.tensor_tensor(
                        out=z[:, zof:zof + heads],
                        in0=as_all[layer][:, blk * heads:(blk + 1) * heads],
                        in1=ad_all[layer][:, blk * heads:(blk + 1) * heads],
                        op=Alu.add)
                zl = spool.tile([128, nzc + nself], f32, tag="zl")
                nc.vector.scalar_tensor_tensor(
                    out=zl[:], in0=z[:], scalar=0.2, in1=z[:],
                    op0=Alu.mult, op1=Alu.max)
                tmpS = wpool.tile([128, ntt, Fo + heads], bf16, tag="tmpS")
                nc.scalar.activation(
                    tmpS[:, :, Fo:Fo + heads],
                    zl[:, :nzc].rearrange("p (t h) -> p t h", t=ntt),
                    Act.Exp)
                ses = spool.tile([128, nself], f32, tag="ses")
                nc.scalar.activation(ses[:], zl[:, nzc:nzc + nself], Act.Exp)
                sv = tmpS[:, :, Fo:Fo + heads]
                for kind in range(2):
                    gk = geo[p][kind]
                    ts0 = 0 if kind == 0 else tkL
                    for hh in range(heads):
                        nc.vector.tensor_tensor(
                            out=tmpS[:, ts0:ts0 + gk["tk"],
                                     hh * HID:(hh + 1) * HID],
                            in0=gtiles[kind][:, :, hh * HID:(hh + 1) * HID],
                            in1=sv[:, ts0:ts0 + gk["tk"], hh:hh + 1]
                                .broadcast_to([128, gk["tk"], HID]),
                            op=Alu.mult)

                # per-block fused (agg | den) + epilogue
                results = []
                for x, blk in blocks:
                    M, ntb = Ms[x]
                    nt_lo, _nt_hi = blk_nt[blk]
                    pagg = ppagg.tile([128, Fo + heads], f32, tag="agg",
                                      space="PSUM")
                    mms = []
                    for kind in range(2):
                        gk = geo[p][kind]
                        t0, t1 = gk["a"] if x == 0 else gk["b"]
                        jbase = 0 if kind == 0 else nt_lo
                        gtb = 0 if kind == 0 else tkL
                        for t in range(t0, t1):
                            mms.append((jbase + t - t0, gtb + t))
                    for i, (j, gt) in enumerate(mms):
                        nc.tensor.matmul(
                            out=pagg[:],
                            lhsT=M[:, j, :],
                            rhs=tmpS[:, gt, :],
                            start=(i == 0), stop=(i == len(mms) - 1))

                    se = spool.tile([128, heads], bf16, tag="se")
                    nc.vector.tensor_tensor(
                        out=se[:], in0=ses[:, x * heads:(x + 1) * heads],
                        in1=mselfc_s[:, blk:blk + 1]
                            .broadcast_to([128, heads]),
                        op=Alu.mult)
                    h_own = wpool.tile([128, Fo], bf16, tag="hown")
                    nc.sync.dma_start(out=h_own[:],
                                      in_=shard[blk * BS:(blk + 1) * BS, :Fo])
                    hs = wpool.tile([128, Fo + heads], bf16, tag="hs")
                    for hh in range(heads):
                        nc.vector.tensor_tensor(
                            out=hs[:, hh * HID:(hh + 1) * HID],
                            in0=h_own[:, hh * HID:(hh + 1) * HID],
                            in1=se[:, hh:hh + 1].broadcast_to([128, HID]),
                            op=Alu.mult)
                    nc.vector.tensor_copy(out=hs[:, Fo:Fo + heads], in_=se[:])

                    t1_ = wpool.tile([128, Fo + heads], f32, tag="t1")
                    nc.vector.tensor_tensor(out=t1_[:], in0=pagg[:],
                                            in1=hs[:], op=Alu.add)
                    den = spool.tile([128, heads], f32, tag="den")
                    nc.vector.tensor_scalar(out=den[:],
                                            in0=t1_[:, Fo:Fo + heads],
                                            scalar1=1e-16, scalar2=None,
                                            op0=Alu.add)
                    rec = spool.tile([128, heads], f32, tag="rec")
                    nc.vector.reciprocal(out=rec[:], in_=den[:])
                    o1 = wpool.tile([128, Fo], f32, tag="o1")
                    nc.vector.tensor_tensor(
                        out=o1[:].rearrange("p (h f) -> p h f", h=heads),
                        in0=t1_[:, :Fo].rearrange("p (h f) -> p h f",
                                                  h=heads),
                        in1=rec[:].unsqueeze(-1)
                            .broadcast_to([128, heads, HID]),
                        op=Alu.mult)
                    o2 = wpool.tile([128, Fo], bf16, tag="o2")
                    nc.vector.tensor_tensor(out=o2[:], in0=o1[:],
                                            in1=bias[:, :Fo], op=Alu.add)
                    if layer == 2:
                        results.append((blk, o2))
                        continue
                    o3 = wpool.tile([128, Fo], bf16, tag="o3")
                    nc.vector.tensor_scalar(out=o3[:], in0=o2[:],
                                            scalar1=0.0, scalar2=None,
                                            op0=Alu.max)
                    hTb = []
                    for k2 in range(2):
                        pt = pptp.tile([128, 128], bf16, tag="tp",
                                       space="PSUM")
                        nc.tensor.transpose(pt[:],
                                            o3[:, k2 * 128:(k2 + 1) * 128],
                                            ident_s[:])
                        ht = spool.tile([128, 128], bf16, tag=f"ht{k2}")
                        nc.vector.tensor_copy(out=ht[:], in_=pt[:])
                        hTb.append(ht)
                    results.append((blk, hTb))
                return results

            # ---- layer 0 transform (batched x^T loads) ----
            for b0_ in range(0, NB, 4):
                nbk = min(4, NB - b0_)
                xb = wpool.tile([IN_C, 4 * BS], bf16, tag="xtb")
                nc.sync.dma_start(
                    out=xb[:, :nbk * BS].rearrange("c (b s) -> c b s", b=nbk),
                    in_=xtb_d[b0_:b0_ + nbk].rearrange("b c s -> c b s"))
                for j in range(nbk):
                    transform_block(0, b0_ + j,
                                    xb[:, j * BS:(j + 1) * BS], None)
            allgather(0)

            # prime gather tiles (stale-tail rows must be finite)
            for _ in range(GBUFS):
                g = gpool.tile([128, max_tk, EL01], bf16, tag="g")
                nc.vector.memset(g[:], 0.0)

            # ---- layer 0/1: aggregate + interleaved next transform ----
            for layer in range(2):
                for p in range(NPAIR):
                    for blk, hTb in agg_pair(layer, p):
                        transform_block(layer + 1, blk, hTb[0][:], hTb[1][:])
                allgather(layer + 1)

            # ---- layer 2: aggregate + masked column sum ----
            psum_sum = ppsum.tile([1, OUT_C], f32, tag="sum", space="PSUM")
            first = True
            for p in range(NPAIR):
                for blk, o2 in agg_pair(2, p):
                    nc.tensor.matmul(out=psum_sum[:],
                                     lhsT=maskc_s[:, blk:blk + 1],
                                     rhs=o2[:], start=first,
                                     stop=(blk == NB - 1))
                    first = False
            osb = spool.tile([1, OUT_C], f32, tag="osb")
            nc.vector.tensor_copy(out=osb[:], in_=psum_sum[:])
            nc.sync.dma_start(out=out_d[:], in_=osb[:])

    nc.compile()
    return nc


def _geo_key(pp):
    return repr([(g[0]["sa"], g[0]["sb"], g[1]["sa"], g[1]["sb"])
                 for g in pp["geo"]])


def _get_built(pp):
    global _BUILT, _BUILT_KEY
    key = _geo_key(pp)
    if _BUILT is None or _BUILT_KEY != key:
        _BUILT = build_kernel(pp["geo"], pp["idx_off"], pp["blk_off"],
                              pp["blk_nt"], pp["idx16"].shape[2],
                              pp["dstc"].shape[2], pp["dstr"].shape[2])
        _BUILT_KEY = key
    return _BUILT


def kernel(**inputs) -> np.ndarray:
    from concourse.bass_utils import run_bass_kernel_spmd

    pp = preprocess(np.asarray(inputs["edge_index"]))
    in_maps = build_core_inputs(inputs, pp)
    nc = _get_built(pp)
    res = run_bass_kernel_spmd(nc, in_maps, core_ids=list(range(NCORES)))
    parts = np.stack([r["out_part"][0] for r in res.results])  # [8, 64]
    g = parts.sum(axis=0, keepdims=True) / N
    out = (g @ np.asarray(inputs["hw"], np.float32)
           + np.asarray(inputs["hb"], np.float32)).astype(np.float32)
    return out


# revision 25
# speedup vs baseline: 1.2329x; 1.0221x over previous
"""3-layer GAT on 8 trn2 NeuronCores.

Strategy (graph/data parallel per sharding hint):
  - Nodes assigned to 8 cores x 49 blocks x 128 slots (degree-balanced LPT
    packing) -> permuted node order; table row = core*6272 + block*128 + slot.
  - Per layer, per node block: transform with rhs = [W | W@as | W@ad] (alpha
    terms folded into the matmul) -> bf16 table shard [6272, 384]; one
    AllGather (Shared output) per layer -> full table on every core.
  - Aggregation processes PAIRS of dst blocks: non-self edges of both blocks
    (dst-sorted) share one dma_gather per int16-index window (lo rows
    [0,32768), hi rows [17408,50176)), amortizing the Q7 descriptor-gen
    fixed cost -- the gather stream is the kernel's critical path.  One-hot
    scatter matrices M [edge,dst] / MT [dst,edge] are built on-device per
    block over its (static) tile range and feed matmuls for the per-edge ad
    term and the fused (feature | denom) accumulation in PSUM.  Self-loop
    contributions are computed from the local shard and never gathered.
  - Softmax max-shift skipped (logits O(1), exp safe; mathematically equal).
  - Next layer's transform is interleaved into the aggregation loop
    (block-level pipeline); layer 2 reduces via a mask matmul; final mean +
    linear head on host.  ACT engine runs only Exp; casts/copies/lrelu/relu
    are DVE ops (ACT copy truncates f32->bf16; DVE rounds).
"""

import os
import numpy as np
import ml_dtypes

# ---------------- problem constants (must match reference) ----------------
N = 50000
E = 800000
IN_C = 128
HID = 64
HEADS = 4
OUT_C = 64
F1 = HEADS * HID  # 256

# ---------------- sharding geometry ----------------
NCORES = 8
NB = 49            # dst blocks per core
BS = 128           # dst slots per block
NPC = NB * BS      # 6272 nodes per core
RTOT = NCORES * NPC  # 50176 table rows
KE_CAP = 1152      # lo/hi packing capacity per (block, kind)
LO_LIM = 32768     # lo window rows [0, 32768)
HI_BASE = 17408    # hi window rows [17408, 50176)
# Aggregation unit: PAIR_BLOCKS=2 shares one dma_gather per window between
# two dst blocks (fewer Q7 fixed costs, but bigger gathers stall the SWDGE
# ring); PAIR_BLOCKS=1 gathers per block (measured faster on HW).
PAIR_BLOCKS = int(os.environ.get("GAT_PAIR", "1"))
NPAIR = (NB + PAIR_BLOCKS - 1) // PAIR_BLOCKS

EL01 = 384         # table elems/row layers 0/1: 256 h + 4 as + 4 ad + pad
EL2 = 128          # table elems/row layer 2: 64 h + 1 as + 1 ad + pad
BF = ml_dtypes.bfloat16

GBUFS = 6          # gather tile double-buffer depth
SINGLE_PACKET = os.environ.get("GAT_SP", "0") == "1"


def _a16(x):
    return (int(x) + 15) // 16 * 16


# ---------------- host preprocessing ----------------

def preprocess(edge_index):
    """Node->(core,block,slot) assignment, pair-packed edge arrays, and the
    static pair geometry consumed by build_kernel."""
    import heapq

    e0 = np.asarray(edge_index[0], np.int64)
    e1 = np.asarray(edge_index[1], np.int64)
    nonself = e0 != e1
    src = e0[nonself]
    dst = e1[nonself]
    # self-edge multiplicity: 1 (PyG added loop) + natural self edges
    mult = np.ones(N, np.int64)
    np.add.at(mult, e0[~nonself], 1)

    deg = np.bincount(dst, minlength=N)  # gather load per dst node

    nblocks = NCORES * NB
    order = np.argsort(-deg, kind="stable")
    heap = [(0, b) for b in range(nblocks)]
    heapq.heapify(heap)
    slots_used = np.zeros(nblocks, np.int64)
    node_block = np.empty(N, np.int64)
    node_slot = np.empty(N, np.int64)
    for n in order:
        while True:
            load, b = heapq.heappop(heap)
            if slots_used[b] < BS:
                break
        node_block[n] = b
        node_slot[n] = slots_used[b]
        slots_used[b] += 1
        heapq.heappush(heap, (load + int(deg[n]), b))

    row = node_block * BS + node_slot  # block-major global table row

    xperm = np.full(RTOT, -1, np.int64)
    xperm[row] = np.arange(N)

    erow = row[src]
    eblk = node_block[dst]
    eslot = node_slot[dst]

    order_e = np.argsort(eblk, kind="stable")
    bounds = np.searchsorted(eblk[order_e], np.arange(nblocks + 1))

    # pass 1: split lo/hi per block, count
    packed = {}
    cnt = np.zeros((NCORES, NB, 2), np.int64)
    for b in range(nblocks):
        c, bl = divmod(b, NB)
        es = order_e[bounds[b]:bounds[b + 1]]
        r_ = erow[es]
        dl = eslot[es]
        lo_f = r_ < HI_BASE
        hi_f = r_ >= LO_LIM
        flex = ~lo_f & ~hi_f
        n_lo = int(lo_f.sum())
        n_hi = int(hi_f.sum())
        n_fx = int(flex.sum())
        tot = n_lo + n_hi + n_fx
        assert tot <= 2 * KE_CAP, f"block {b} has {tot} edges > {2*KE_CAP}"
        add_lo = min(n_fx, max(0, min(KE_CAP, (tot + 1) // 2) - n_lo))
        if n_hi + (n_fx - add_lo) > KE_CAP:
            add_lo = n_fx - (KE_CAP - n_hi)
        assert 0 <= add_lo <= n_fx
        fx_idx = np.nonzero(flex)[0]
        sel_lo = np.zeros(len(es), bool)
        sel_lo[lo_f] = True
        sel_lo[fx_idx[:add_lo]] = True
        for kind, sel, base in ((0, sel_lo, 0), (1, ~sel_lo, HI_BASE)):
            rr = r_[sel]
            dd = dl[sel]
            o = np.argsort(rr, kind="stable")  # DMA locality
            packed[(b, kind)] = (rr[o] - base, dd[o])
            cnt[c, bl, kind] = len(rr)

    # ---- static pair geometry ----
    # pair p = blocks (2p, 2p+1); per (pair, kind): section sizes sa/sb
    # (16-aligned max over cores), gather reg = sa+sb, tiles tk.
    # Per block and kind: tile range [t0, t1) within the pair's tiles.
    geo = []
    for p in range(NPAIR):
        ba = PAIR_BLOCKS * p
        bb = ba + 1
        has_b = PAIR_BLOCKS == 2 and bb < NB
        pk = []
        for kind in range(2):
            sa = _a16(cnt[:, ba, kind].max())
            sb = _a16(cnt[:, bb, kind].max()) if has_b else 0
            skp = sa + sb
            tk = (skp + 127) // 128
            a0, a1 = 0, (sa + 127) // 128
            b0, b1 = sa // 128, tk
            pk.append(dict(sa=sa, sb=sb, skp=skp, tk=tk,
                           a=(a0, a1), b=(b0, b1) if has_b else None))
        geo.append(pk)

    # flat column offsets for idx16 / per-block dstc (dstcX: per block, both
    # kinds adjacent: [lo tiles | hi tiles] over that block's tile ranges)
    idx_off = []
    o = 0
    for p in range(NPAIR):
        idx_off.append(o)
        o += (geo[p][0]["tk"] * 128 + geo[p][1]["tk"] * 128) // 16
    idx_cols = o

    blk_off = []   # per block: column offset into dstc flat (tile units)
    blk_nt = []    # per block: (nt_lo, nt_hi)
    o = 0
    for bl in range(NB):
        p, x = divmod(bl, PAIR_BLOCKS)
        rng = [geo[p][k]["a" if x == 0 else "b"] for k in range(2)]
        nt = [r[1] - r[0] for r in rng]
        blk_off.append(o)
        blk_nt.append(tuple(nt))
        o += nt[0] + nt[1]
    dstc_cols = o

    dstr_cols = max((blk_nt[b][0] + blk_nt[b][1]) * 128 for b in range(NB))
    idx16 = np.full((NCORES, 128, idx_cols), -1, np.int16)
    dstc = np.full((NCORES, 128, dstc_cols), -1.0, np.float32)
    dstr = np.full((NCORES, NB, dstr_cols), -1.0, np.float32)
    maskc = np.zeros((NCORES, 128, NB), np.float32)
    mselfc = np.zeros((NCORES, 128, NB), np.float32)

    for c in range(NCORES):
        for p in range(NPAIR):
            ba = PAIR_BLOCKS * p
            bb = ba + 1
            has_b = PAIR_BLOCKS == 2 and bb < NB
            col0 = idx_off[p]
            for kind in range(2):
                gk = geo[p][kind]
                sa, sb, skp, tk = gk["sa"], gk["sb"], gk["skp"], gk["tk"]
                kep = tk * 128
                relA, ddA = packed[(c * NB + ba, kind)]
                full = np.full(kep, -1, np.int64)
                dloc = np.full(kep, -1.0, np.float32)
                kA = len(relA)
                full[:kA] = relA
                full[kA:sa] = 0
                dloc[:kA] = ddA.astype(np.float32)
                if has_b:
                    relB, ddB = packed[(c * NB + bb, kind)]
                    kB = len(relB)
                    full[sa:sa + kB] = relB
                    full[sa + kB:skp] = 0
                    dloc[sa:sa + kB] = ddB.astype(np.float32)
                w = full.reshape(kep // 16, 16).T.astype(np.int16)
                idx16[c, :, col0:col0 + kep // 16] = np.tile(w, (8, 1))
                col0 += kep // 16
                # per-block dstc over tile ranges (other block's slots = -1)
                for x, blk in ((0, ba),) + (((1, bb),) if has_b else ()):
                    t0, t1 = gk["a"] if x == 0 else gk["b"]
                    dl = dloc.copy()
                    if x == 0:
                        dl[sa:] = -1.0
                    else:
                        dl[:sa] = -1.0
                    seg = dl.reshape(tk, 128)[t0:t1]          # [nt, 128]
                    off = blk_off[blk] + (0 if kind == 0 else blk_nt[blk][0])
                    nt = t1 - t0
                    dstc[c, :, off:off + nt] = seg.T
                    dstr[c, blk, (0 if kind == 0 else
                                  blk_nt[blk][0] * 128):][:nt * 128] = \
                        seg.reshape(-1)

        for bl in range(NB):
            b = c * NB + bl
            used = slots_used[b]
            maskc[c, :used, bl] = 1.0
            nodes_b = np.where(node_block == b)[0]
            mselfc[c, node_slot[nodes_b], bl] = \
                mult[nodes_b].astype(np.float32)

    return dict(row=row, xperm=xperm, idx16=idx16, dstc=dstc, dstr=dstr,
                maskc=maskc, mselfc=mselfc, cnt=cnt, geo=geo,
                idx_off=idx_off, blk_off=blk_off, blk_nt=blk_nt,
                node_block=node_block, node_slot=node_slot)


def host_weights(inputs):
    """Extended weight matrices with folded attention vectors."""
    def ext(W, a_s, a_d, heads):
        Wh = W.reshape(W.shape[0], heads, HID)
        Was = np.einsum("khc,hc->kh", Wh, a_s)
        Wad = np.einsum("khc,hc->kh", Wh, a_d)
        return np.concatenate([W, Was, Wad], axis=1).astype(np.float32)

    W0e = ext(np.asarray(inputs["W0"], np.float32),
              np.asarray(inputs["a0s"], np.float32),
              np.asarray(inputs["a0d"], np.float32), HEADS)      # [128, 264]
    W1e = ext(np.asarray(inputs["W1"], np.float32),
              np.asarray(inputs["a1s"], np.float32),
              np.asarray(inputs["a1d"], np.float32), HEADS)      # [256, 264]
    W2e = ext(np.asarray(inputs["W2"], np.float32),
              np.asarray(inputs["a2s"], np.float32),
              np.asarray(inputs["a2d"], np.float32), 1)          # [256, 66]
    return W0e, W1e, W2e


def build_core_inputs(inputs, pp):
    """Per-core in_maps for run_bass_kernel_spmd."""
    x = np.asarray(inputs["x"], np.float32)
    W0e, W1e, W2e = host_weights(inputs)
    b0 = np.asarray(inputs["b0"], np.float32)
    b1 = np.asarray(inputs["b1"], np.float32)
    b2 = np.asarray(inputs["b2"], np.float32)

    iota_row = np.tile(np.arange(128, dtype=np.float32), (128, 1))
    iota_col = np.arange(128, dtype=np.float32).reshape(128, 1)
    ones1 = np.ones((1, 128), np.float32)
    ident = np.eye(128, dtype=np.float32)

    consts = dict(
        w0e=W0e.astype(BF),
        w1e=W1e.reshape(2, 128, F1 + 2 * HEADS).astype(BF),
        w2e=W2e.reshape(2, 128, HID + 2).astype(BF),
        b0r=np.tile(b0, (128, 1)).astype(BF),
        b1r=np.tile(b1, (128, 1)).astype(BF),
        b2r=np.tile(b2, (128, 1)).astype(BF),
        iota_row=iota_row.astype(BF), iota_col=iota_col.astype(BF),
        ones1=ones1.astype(BF), ident=ident.astype(BF),
    )

    in_maps = []
    for c in range(NCORES):
        xtb = np.zeros((NB, IN_C, BS), np.float32)
        rows = np.arange(c * NPC, (c + 1) * NPC)
        nodes = pp["xperm"][rows].reshape(NB, BS)
        for b in range(NB):
            nb = nodes[b]
            valid = nb >= 0
            if valid.any():
                xtb[b][:, valid] = x[nb[valid]].T
        m = dict(
            xtb=xtb.astype(BF),
            idx16=pp["idx16"][c],
            dstc=pp["dstc"][c].astype(BF),
            dstr=pp["dstr"][c].astype(BF),
            maskc=pp["maskc"][c].astype(BF),
            mselfc=pp["mselfc"][c].astype(BF),
            **consts,
        )
        in_maps.append(m)
    return in_maps


# ---------------- numpy emulation of the device data path ----------------

def _emulate_layer(tables_in, pp, We, bias, heads, Fo, relu, el):
    """tables_in: node-major feature mat [RTOT, F_in] (f32).
    Mirrors the pair-packed device data path (reads pp's flat arrays)."""
    geo = pp["geo"]
    ncols = Fo + 2 * heads
    tb = (tables_in.astype(BF).astype(np.float32)
          @ We.astype(BF).astype(np.float32))
    table = np.zeros((RTOT, el), BF)
    table[:, :ncols] = tb.astype(BF)
    as_all = tb[:, Fo:Fo + heads].astype(BF).astype(np.float32)
    ad_all = tb[:, Fo + heads:Fo + 2 * heads].astype(BF).astype(np.float32)

    def lrexp(z):
        return np.exp(np.maximum(z, 0.2 * z)).astype(np.float32)

    out = np.zeros((RTOT, Fo), np.float32)
    for c in range(NCORES):
        for p in range(len(geo)):
            col0 = pp["idx_off"][p]
            gs = []
            dls = []
            for kind in range(2):
                gk = geo[p][kind]
                kep = gk["tk"] * 128
                w = pp["idx16"][c][:16, col0:col0 + kep // 16]
                col0 += kep // 16
                rel = w.T.reshape(-1).astype(np.int64)
                r = gk["skp"]
                base = 0 if kind == 0 else HI_BASE
                rows = rel[:r] + base
                g = np.zeros((kep, el), np.float32)
                g[:r] = np.asarray(table[rows], np.float32)
                gs.append(g)
            ba = PAIR_BLOCKS * p
            blks = ((0, ba),) + (((1, ba + 1),)
                                 if PAIR_BLOCKS == 2 and ba + 1 < NB else ())
            for x, blk in blks:
                bl = blk
                rbase = c * NPC + bl * BS
                agg = np.zeros((BS, Fo), np.float32)
                den = np.zeros((BS, heads), np.float32)
                for kind in range(2):
                    gk = geo[p][kind]
                    t0, t1 = gk["a"] if x == 0 else gk["b"]
                    off = pp["blk_off"][bl] + (
                        0 if kind == 0 else pp["blk_nt"][bl][0])
                    nt = t1 - t0
                    dl = pp["dstc"][c][:, off:off + nt].T.reshape(-1)
                    dl = dl.astype(np.int64)                    # [nt*128]
                    g = gs[kind][t0 * 128:t1 * 128]             # [nt*128, el]
                    valid = dl >= 0
                    a_s = g[:, Fo:Fo + heads]
                    a_d = np.where(valid[:, None],
                                   ad_all[rbase + dl % 128 if False else
                                          rbase + np.clip(dl, 0, None)], 0.0)
                    s = lrexp(a_s + a_d).astype(BF).astype(np.float32)
                    s = np.where(valid[:, None], s, 0.0)
                    hsc = (g[:, :Fo].reshape(-1, heads, HID)
                           * s[:, :, None]).astype(BF).astype(np.float32)
                    hsc = hsc.reshape(-1, Fo)
                    np.add.at(agg, np.clip(dl, 0, None)[valid], hsc[valid])
                    np.add.at(den, np.clip(dl, 0, None)[valid], s[valid])
                # self loops
                ms = pp["mselfc"][c][:, bl]
                ss = lrexp(as_all[rbase:rbase + BS]
                           + ad_all[rbase:rbase + BS])
                se = (ss * ms[:, None]).astype(BF).astype(np.float32)
                h_own = np.asarray(table[rbase:rbase + BS, :Fo], np.float32)
                hs = (h_own.reshape(BS, heads, HID)
                      * se[:, :, None]).astype(BF).astype(np.float32)
                agg += hs.reshape(BS, Fo)
                den += se
                o = agg.reshape(BS, heads, HID) / (den + 1e-16)[:, :, None]
                o = o.reshape(BS, Fo) + bias
                if relu:
                    o = np.maximum(o, 0.0)
                out[rbase:rbase + BS] = o
    return out


def emulate(inputs, pp=None):
    """Full numpy emulation; returns [1, OUT_C]."""
    if pp is None:
        pp = preprocess(np.asarray(inputs["edge_index"]))
    x = np.asarray(inputs["x"], np.float32)
    W0e, W1e, W2e = host_weights(inputs)
    h = np.zeros((RTOT, IN_C), np.float32)
    valid = pp["xperm"] >= 0
    h[valid] = x[pp["xperm"][valid]]

    b0 = np.asarray(inputs["b0"], np.float32)
    b1 = np.asarray(inputs["b1"], np.float32)
    b2 = np.asarray(inputs["b2"], np.float32)

    h0 = _emulate_layer(h, pp, W0e, b0, HEADS, F1, True, EL01)
    h1 = _emulate_layer(h0, pp, W1e, b1, HEADS, F1, True, EL01)
    h2 = _emulate_layer(h1, pp, W2e, b2, 1, HID, False, EL2)

    g = h2[valid].sum(axis=0, keepdims=True) / N
    return (g @ np.asarray(inputs["hw"], np.float32)
            + np.asarray(inputs["hb"], np.float32)).astype(np.float32)


# ---------------- device kernel ----------------

_BUILT = None
_BUILT_KEY = None


def build_kernel(geo, idx_off, blk_off, blk_nt, idx_cols, dstc_cols,
                 dstr_cols):
    import concourse.bacc as bacc
    import concourse.mybir as mybir
    import concourse.tile as tile
    from concourse import library_config

    f32 = mybir.dt.float32
    bf16 = mybir.dt.bfloat16
    i16 = mybir.dt.int16
    Alu = mybir.AluOpType
    Act = mybir.ActivationFunctionType

    nc = bacc.Bacc("TRN2", target_bir_lowering=False, debug=False,
                   num_devices=NCORES, num_swdge_queues=2)

    max_tk = max(g[k]["tk"] for g in geo for k in range(2))
    max_nt = max(nt[0] + nt[1] for nt in blk_nt)

    # ---- I/O ----
    xtb_d = nc.dram_tensor("xtb", [NB, IN_C, BS], bf16, kind="ExternalInput")
    idx16_d = nc.dram_tensor("idx16", [128, idx_cols], i16,
                             kind="ExternalInput")
    dstc_d = nc.dram_tensor("dstc", [128, dstc_cols], bf16,
                            kind="ExternalInput")
    dstr_d = nc.dram_tensor("dstr", [NB, dstr_cols], bf16,
                            kind="ExternalInput")
    maskc_d = nc.dram_tensor("maskc", [128, NB], bf16, kind="ExternalInput")
    mselfc_d = nc.dram_tensor("mselfc", [128, NB], bf16, kind="ExternalInput")
    w0e_d = nc.dram_tensor("w0e", [IN_C, F1 + 2 * HEADS], bf16,
                           kind="ExternalInput")
    w1e_d = nc.dram_tensor("w1e", [2, 128, F1 + 2 * HEADS], bf16,
                           kind="ExternalInput")
    w2e_d = nc.dram_tensor("w2e", [2, 128, HID + 2], bf16,
                           kind="ExternalInput")
    b0r_d = nc.dram_tensor("b0r", [128, F1], bf16, kind="ExternalInput")
    b1r_d = nc.dram_tensor("b1r", [128, F1], bf16, kind="ExternalInput")
    b2r_d = nc.dram_tensor("b2r", [128, HID], bf16, kind="ExternalInput")
    iota_row_d = nc.dram_tensor("iota_row", [128, 128], bf16,
                                kind="ExternalInput")
    iota_col_d = nc.dram_tensor("iota_col", [128, 1], bf16,
                                kind="ExternalInput")
    ones1_d = nc.dram_tensor("ones1", [1, 128], bf16, kind="ExternalInput")
    ident_d = nc.dram_tensor("ident", [128, 128], bf16, kind="ExternalInput")
    out_d = nc.dram_tensor("out_part", [1, OUT_C], f32, kind="ExternalOutput")

    # internal DRAM
    tables = []
    shards = []
    for li, el in enumerate([EL01, EL01, EL2]):
        tables.append(nc.dram_tensor(f"table{li}", [RTOT, el], bf16,
                                     addr_space="Shared"))
        shards.append(nc.dram_tensor(f"shard{li}", [NPC, el], bf16))

    rg = [list(range(NCORES))]

    with tile.TileContext(nc) as tc:
        with (
            tc.tile_pool(name="const", bufs=1) as cpool,
            tc.tile_pool(name="gather", bufs=GBUFS) as gpool,
            tc.tile_pool(name="onehot", bufs=4) as mpool,
            tc.tile_pool(name="work", bufs=3) as wpool,
            tc.tile_pool(name="small", bufs=4) as spool,
            tc.tile_pool(name="adas", bufs=1) as apool,
            tc.tile_pool(name="ps_agg", bufs=2, space="PSUM") as ppagg,
            tc.tile_pool(name="ps_pad", bufs=2, space="PSUM") as pppad,
            tc.tile_pool(name="ps_rep", bufs=1, space="PSUM") as pprep,
            tc.tile_pool(name="ps_tp", bufs=1, space="PSUM") as pptp,
            tc.tile_pool(name="ps_tf", bufs=1, space="PSUM") as pptf,
            tc.tile_pool(name="ps_sum", bufs=1, space="PSUM") as ppsum,
        ):
            def load_const(tag, dram, shape, dtype=bf16, view=None):
                t = cpool.tile(shape, dtype, tag=tag)
                nc.sync.dma_start(out=t[:], in_=view if view is not None
                                  else dram[:])
                return t

            w0e_s = load_const("w0e", w0e_d, [IN_C, F1 + 2 * HEADS])
            w1e_s = load_const("w1e", w1e_d, [128, 2, F1 + 2 * HEADS],
                               view=w1e_d[:].rearrange("c p j -> p c j"))
            w2e_s = load_const("w2e", w2e_d, [128, 2, HID + 2],
                               view=w2e_d[:].rearrange("c p j -> p c j"))
            b0r_s = load_const("b0r", b0r_d, [128, F1])
            b1r_s = load_const("b1r", b1r_d, [128, F1])
            b2r_s = load_const("b2r", b2r_d, [128, HID])
            iota_row_s = load_const("iota_row", iota_row_d, [128, 128])
            iota_col_s = load_const("iota_col", iota_col_d, [128, 1])
            ones1_s = load_const("ones1", ones1_d, [1, 128])
            ident_s = load_const("ident", ident_d, [128, 128])
            idx16_s = load_const("idx16", idx16_d, [128, idx_cols], i16)
            dstc_s = load_const("dstc", dstc_d, [128, dstc_cols])
            maskc_s = load_const("maskc", maskc_d, [128, NB])
            mselfc_s = load_const("mselfc", mselfc_d, [128, NB])

            nc.gpsimd.load_library(library_config.mlp)

            # persistent per-layer alpha tiles [128, NB*heads]
            as_all0 = apool.tile([128, NB * HEADS], bf16, tag="as0")
            as_all1 = apool.tile([128, NB * HEADS], bf16, tag="as1")
            as_all2 = apool.tile([128, NB], bf16, tag="as2")
            ad_all0 = apool.tile([128, NB * HEADS], bf16, tag="ad0")
            ad_all1 = apool.tile([128, NB * HEADS], bf16, tag="ad1")
            ad_all2 = apool.tile([128, NB], bf16, tag="ad2")
            as_all = [as_all0, as_all1, as_all2]
            ad_all = [ad_all0, ad_all1, ad_all2]

            LCFG = [  # heads, Fo, ncols, el, bias, relu
                (HEADS, F1, F1 + 2 * HEADS, EL01, b0r_s, True),
                (HEADS, F1, F1 + 2 * HEADS, EL01, b1r_s, True),
                (1, HID, HID + 2, EL2, b2r_s, False),
            ]

            def transform_block(layer, b, lhsT0, lhsT1):
                heads, Fo, ncols, el, _bias, _relu = LCFG[layer]
                shard = shards[layer]
                ps = pptf.tile([128, 512], f32, tag="tf", space="PSUM")
                if layer == 0:
                    nc.tensor.matmul(out=ps[:, :ncols], lhsT=lhsT0,
                                     rhs=w0e_s[:], start=True, stop=True)
                else:
                    we = w1e_s if layer == 1 else w2e_s
                    nc.tensor.matmul(out=ps[:, :ncols], lhsT=lhsT0,
                                     rhs=we[:, 0, :], start=True, stop=False)
                    nc.tensor.matmul(out=ps[:, :ncols], lhsT=lhsT1,
                                     rhs=we[:, 1, :], start=False, stop=True)
                tb = wpool.tile([128, EL01], bf16, tag="tb")
                nc.vector.tensor_copy(out=tb[:, :ncols], in_=ps[:, :ncols])
                nc.vector.tensor_copy(
                    out=as_all[layer][:, b * heads:(b + 1) * heads],
                    in_=ps[:, Fo:Fo + heads])
                nc.vector.tensor_copy(
                    out=ad_all[layer][:, b * heads:(b + 1) * heads],
                    in_=ps[:, Fo + heads:Fo + 2 * heads])
                nc.sync.dma_start(out=shard[b * BS:(b + 1) * BS, :],
                                  in_=tb[:, :el])

            def allgather(layer):
                nc.gpsimd.collective_compute(
                    "AllGather", mybir.AluOpType.bypass,
                    replica_groups=rg, ins=[shards[layer][:].opt()],
                    outs=[tables[layer][:].opt()])

            def agg_pair(layer, p):
                """Aggregate blocks (2p, 2p+1); returns per-block results."""
                heads, Fo, ncols, el, bias, relu = LCFG[layer]
                table = tables[layer]
                shard = shards[layer]
                views = [table[0:LO_LIM, :], table[HI_BASE:HI_BASE + 32768, :]]
                gA, gB = geo[p][0], geo[p][1]
                tkL, tkH = gA["tk"], gB["tk"]
                ntt = tkL + tkH
                ba = PAIR_BLOCKS * p
                blocks = [(0, ba)] + ([(1, ba + 1)]
                                      if PAIR_BLOCKS == 2 and ba + 1 < NB
                                      else [])

                # paired gathers (critical Q7 stream)
                gtiles = []
                col0 = idx_off[p]
                for kind in range(2):
                    gk = geo[p][kind]
                    kep = gk["tk"] * 128
                    g = gpool.tile([128, gk["tk"], el], bf16, tag="g")
                    nc.gpsimd.dma_gather(
                        g[:], views[kind],
                        idx16_s[:, col0:col0 + kep // 16],
                        kep, gk["skp"], el,
                        single_packet=SINGLE_PACKET, queue_num=kind)
                    col0 += kep // 16
                    gtiles.append(g)

                # per-block one-hot M/MT + adp into the shared pair pad_
                pad_ = pppad.tile([128, ntt * heads], f32, tag="adp",
                                  space="PSUM")
                Ms = {}

                def tile_writers(gt):
                    """Blocks covering pair-tile gt (for adp start/stop)."""
                    kind = 0 if gt < tkL else 1
                    t = gt - (0 if kind == 0 else tkL)
                    gk = geo[p][kind]
                    ws = []
                    for x, blk in blocks:
                        rng = gk["a"] if x == 0 else gk["b"]
                        if rng[0] <= t < rng[1]:
                            ws.append(x)
                    return ws

                for x, blk in blocks:
                    nt_lo, nt_hi = blk_nt[blk]
                    ntb = nt_lo + nt_hi
                    off = blk_off[blk]
                    M = mpool.tile([128, max_nt, 128], bf16, tag="M")
                    nc.vector.tensor_tensor(
                        out=M[:, :ntb, :],
                        in0=dstc_s[:, off:off + ntb].unsqueeze(-1)
                            .broadcast_to([128, ntb, 128]),
                        in1=iota_row_s[:].unsqueeze(1)
                            .broadcast_to([128, ntb, 128]),
                        op=Alu.is_equal)
                    Ms[x] = (M, ntb)
                    MT = mpool.tile([128, max_nt * 128], bf16, tag="MT")
                    dr = spool.tile([1, max_nt * 128], bf16, tag="dr")
                    nc.sync.dma_start(out=dr[:, :ntb * 128],
                                      in_=dstr_d[blk:blk + 1, :ntb * 128])
                    for o in range(0, ntb * 128, 512):
                        wd = min(512, ntb * 128 - o)
                        pr = pprep.tile([128, 512], f32, tag="rep",
                                        space="PSUM")
                        nc.tensor.matmul(out=pr[:, :wd], lhsT=ones1_s[:],
                                         rhs=dr[:, o:o + wd],
                                         start=True, stop=True)
                        nc.vector.tensor_tensor(
                            out=MT[:, o:o + wd], in0=pr[:, :wd],
                            in1=iota_col_s[:].broadcast_to([128, wd]),
                            op=Alu.is_equal)
                    # adp matmuls over this block's tiles
                    for kind in range(2):
                        gk = geo[p][kind]
                        t0, t1 = gk["a"] if x == 0 else gk["b"]
                        jbase = 0 if kind == 0 else nt_lo
                        gtb = 0 if kind == 0 else tkL
                        for t in range(t0, t1):
                            gt = gtb + t
                            ws = tile_writers(gt)
                            nc.tensor.matmul(
                                out=pad_[:, gt * heads:(gt + 1) * heads],
                                lhsT=MT[:, (jbase + t - t0) * 128:
                                        (jbase + t - t0 + 1) * 128],
                                rhs=ad_all[layer][:,
                                                  blk * heads:
                                                  (blk + 1) * heads],
                                start=(ws[0] == x), stop=(ws[-1] == x))

                # z for all pair tiles + self-z tails (one group per block)
                nzc = ntt * heads
                nself = len(blocks) * heads
                z = spool.tile([128, nzc + nself], f32, tag="z")
                for kind in range(2):
                    gk = geo[p][kind]
                    zof = (0 if kind == 0 else tkL) * heads
                    nc.vector.tensor_tensor(
                        out=z[:, zof:zof + gk["tk"] * heads]
                            .rearrange("p (t h) -> p t h", t=gk["tk"]),
                        in0=gtiles[kind][:, :, Fo:Fo + heads],
                        in1=pad_[:, zof:zof + gk["tk"] * heads]
                            .rearrange("p (t h) -> p t h", t=gk["tk"]),
                        op=Alu.add)
                for x, blk in blocks:
                    zof = nzc + x * heads
                    nc.vector.tensor_tensor(
                        out=z[:, zof:zof + heads],
                        in0=as_all[layer][:, blk * heads:(blk + 1) * heads],
                        in1=ad_all[layer][:, blk * heads:(blk + 1) * heads],
                        op=Alu.add)
                zl = spool.tile([128, nzc + nself], f32, tag="zl")
                nc.vector.scalar_tensor_tensor(
                    out=zl[:], in0=z[:], scalar=0.2, in1=z[:],
                    op0=Alu.mult, op1=Alu.max)
                tmpS = wpool.tile([128, ntt, Fo + heads], bf16, tag="tmpS")
                nc.scalar.activation(
                    tmpS[:, :, Fo:Fo + heads],
                    zl[:, :nzc].rearrange("p (t h) -> p t h", t=ntt),
                    Act.Exp)
                ses = spool.tile([128, nself], f32, tag="ses")
                nc.scalar.activation(ses[:], zl[:, nzc:nzc + nself], Act.Exp)
                sv = tmpS[:, :, Fo:Fo + heads]
                for kind in range(2):
                    gk = geo[p][kind]
                    ts0 = 0 if kind == 0 else tkL
                    for hh in range(heads):
                        nc.vector.tensor_tensor(
                            out=tmpS[:, ts0:ts0 + gk["tk"],
                                     hh * HID:(hh + 1) * HID],
                            in0=gtiles[kind][:, :, hh * HID:(hh + 1) * HID],
                            in1=sv[:, ts0:ts0 + gk["tk"], hh:hh + 1]
                                .broadcast_to([128, gk["tk"], HID]),
                            op=Alu.mult)

                # per-block fused (agg | den) + epilogue
                results = []
                for x, blk in blocks:
                    M, ntb = Ms[x]
                    nt_lo, _nt_hi = blk_nt[blk]
                    pagg = ppagg.tile([128, Fo + heads], f32, tag="agg",
                                      space="PSUM")
                    mms = []
                    for kind in range(2):
                        gk = geo[p][kind]
                        t0, t1 = gk["a"] if x == 0 else gk["b"]
                        jbase = 0 if kind == 0 else nt_lo
                        gtb = 0 if kind == 0 else tkL
                        for t in range(t0, t1):
                            mms.append((jbase + t - t0, gtb + t))
                    for i, (j, gt) in enumerate(mms):
                        nc.tensor.matmul(
                            out=pagg[:],
                            lhsT=M[:, j, :],
                            rhs=tmpS[:, gt, :],
                            start=(i == 0), stop=(i == len(mms) - 1))

                    se = spool.tile([128, heads], bf16, tag="se")
                    nc.vector.tensor_tensor(
                        out=se[:], in0=ses[:, x * heads:(x + 1) * heads],
                        in1=mselfc_s[:, blk:blk + 1]
                            .broadcast_to([128, heads]),
                        op=Alu.mult)
                    h_own = wpool.tile([128, Fo], bf16, tag="hown")
                    nc.sync.dma_start(out=h_own[:],
                                      in_=shard[blk * BS:(blk + 1) * BS, :Fo])
                    hs = wpool.tile([128, Fo + heads], bf16, tag="hs")
                    for hh in range(heads):
                        nc.vector.tensor_tensor(
                            out=hs[:, hh * HID:(hh + 1) * HID],
                            in0=h_own[:, hh * HID:(hh + 1) * HID],
                            in1=se[:, hh:hh + 1].broadcast_to([128, HID]),
                            op=Alu.mult)
                    nc.vector.tensor_copy(out=hs[:, Fo:Fo + heads], in_=se[:])

                    t1_ = wpool.tile([128, Fo + heads], f32, tag="t1")
                    nc.vector.tensor_tensor(out=t1_[:], in0=pagg[:],
                                            in1=hs[:], op=Alu.add)
                    den = spool.tile([128, heads], f32, tag="den")
                    nc.vector.tensor_scalar(out=den[:],
                                            in0=t1_[:, Fo:Fo + heads],
                                            scalar1=1e-16, scalar2=None,
                                            op0=Alu.add)
                    rec = spool.tile([128, heads], f32, tag="rec")
                    nc.vector.reciprocal(out=rec[:], in_=den[:])
                    o1 = wpool.tile([128, Fo], f32, tag="o1")
                    nc.vector.tensor_tensor(
                        out=o1[:].rearrange("p (h f) -> p h f", h=heads),
                        in0=t1_[:, :Fo].rearrange("p (h f) -> p h f",
                                                  h=heads),
                        in1=rec[:].unsqueeze(-1)
                            .broadcast_to([128, heads, HID]),
                        op=Alu.mult)
                    o2 = wpool.tile([128, Fo], bf16, tag="o2")
                    nc.vector.tensor_tensor(out=o2[:], in0=o1[:],
                                            in1=bias[:, :Fo], op=Alu.add)
                    if layer == 2:
                        results.append((blk, o2))
                        continue
                    o3 = wpool.tile([128, Fo], bf16, tag="o3")
                    nc.scalar.activation(o3[:], o2[:], Act.Relu)
                    hTb = []
                    for k2 in range(2):
                        pt = pptp.tile([128, 128], bf16, tag="tp",
                                       space="PSUM")
                        nc.tensor.transpose(pt[:],
                                            o3[:, k2 * 128:(k2 + 1) * 128],
                                            ident_s[:])
                        ht = spool.tile([128, 128], bf16, tag=f"ht{k2}")
                        nc.vector.tensor_copy(out=ht[:], in_=pt[:])
                        hTb.append(ht)
                    results.append((blk, hTb))
                return results

            # ---- layer 0 transform (batched x^T loads) ----
            for b0_ in range(0, NB, 4):
                nbk = min(4, NB - b0_)
                xb = wpool.tile([IN_C, 4 * BS], bf16, tag="xtb")
                nc.sync.dma_start(
                    out=xb[:, :nbk * BS].rearrange("c (b s) -> c b s", b=nbk),
                    in_=xtb_d[b0_:b0_ + nbk].rearrange("b c s -> c b s"))
                for j in range(nbk):
                    transform_block(0, b0_ + j,
                                    xb[:, j * BS:(j + 1) * BS], None)
            allgather(0)

            # prime gather tiles (stale-tail rows must be finite)
            for _ in range(GBUFS):
                g = gpool.tile([128, max_tk, EL01], bf16, tag="g")
                nc.vector.memset(g[:], 0.0)

            # ---- layer 0/1: aggregate + interleaved next transform ----
            for layer in range(2):
                for p in range(NPAIR):
                    for blk, hTb in agg_pair(layer, p):
                        transform_block(layer + 1, blk, hTb[0][:], hTb[1][:])
                allgather(layer + 1)

            # ---- layer 2: aggregate + masked column sum ----
            psum_sum = ppsum.tile([1, OUT_C], f32, tag="sum", space="PSUM")
            first = True
            for p in range(NPAIR):
                for blk, o2 in agg_pair(2, p):
                    nc.tensor.matmul(out=psum_sum[:],
                                     lhsT=maskc_s[:, blk:blk + 1],
                                     rhs=o2[:], start=first,
                                     stop=(blk == NB - 1))
                    first = False
            osb = spool.tile([1, OUT_C], f32, tag="osb")
            nc.vector.tensor_copy(out=osb[:], in_=psum_sum[:])
            nc.sync.dma_start(out=out_d[:], in_=osb[:])

    nc.compile()
    return nc


def _geo_key(pp):
    return repr([(g[0]["sa"], g[0]["sb"], g[1]["sa"], g[1]["sb"])
                 for g in pp["geo"]])


def _get_built(pp):
    global _BUILT, _BUILT_KEY
    key = _geo_key(pp)
    if _BUILT is None or _BUILT_KEY != key:
        _BUILT = build_kernel(pp["geo"], pp["idx_off"], pp["blk_off"],
                              pp["blk_nt"], pp["idx16"].shape[2],
                              pp["dstc"].shape[2], pp["dstr"].shape[2])
        _BUILT_KEY = key
    return _BUILT


def kernel(**inputs) -> np.ndarray:
    from concourse.bass_utils import run_bass_kernel_spmd

    pp = preprocess(np.asarray(inputs["edge_index"]))
    in_maps = build_core_inputs(inputs, pp)
    nc = _get_built(pp)
    res = run_bass_kernel_spmd(nc, in_maps, core_ids=list(range(NCORES)))
    parts = np.stack([r["out_part"][0] for r in res.results])  # [8, 64]
    g = parts.sum(axis=0, keepdims=True) / N
    out = (g @ np.asarray(inputs["hw"], np.float32)
           + np.asarray(inputs["hb"], np.float32)).astype(np.float32)
    return out


# revision 27
# speedup vs baseline: 1.2616x; 1.0233x over previous
"""3-layer GAT on 8 trn2 NeuronCores.

Strategy (graph/data parallel per sharding hint):
  - Nodes assigned to 8 cores x 49 blocks x 128 slots (degree-balanced LPT
    packing) -> permuted node order; table row = core*6272 + block*128 + slot.
  - Per layer, per node block: transform with rhs = [W | W@as | W@ad] (alpha
    terms folded into the matmul) -> bf16 table shard [6272, 384]; one
    AllGather (Shared output) per layer -> full table on every core.
  - Aggregation processes PAIRS of dst blocks: non-self edges of both blocks
    (dst-sorted) share one dma_gather per int16-index window (lo rows
    [0,32768), hi rows [17408,50176)), amortizing the Q7 descriptor-gen
    fixed cost -- the gather stream is the kernel's critical path.  One-hot
    scatter matrices M [edge,dst] / MT [dst,edge] are built on-device per
    block over its (static) tile range and feed matmuls for the per-edge ad
    term and the fused (feature | denom) accumulation in PSUM.  Self-loop
    contributions are computed from the local shard and never gathered.
  - Softmax max-shift skipped (logits O(1), exp safe; mathematically equal).
  - Next layer's transform is interleaved into the aggregation loop
    (block-level pipeline); layer 2 reduces via a mask matmul; final mean +
    linear head on host.  ACT engine runs Exp and Relu only; casts/copies/
    lrelu are DVE ops (ACT copy truncates f32->bf16; DVE rounds).  Gathers
    alternate two SWDGE queues so descriptor generation and ring drain
    pipeline instead of serializing.
"""

import os
import numpy as np
import ml_dtypes

# ---------------- problem constants (must match reference) ----------------
N = 50000
E = 800000
IN_C = 128
HID = 64
HEADS = 4
OUT_C = 64
F1 = HEADS * HID  # 256

# ---------------- sharding geometry ----------------
NCORES = 8
NB = 49            # dst blocks per core
BS = 128           # dst slots per block
NPC = NB * BS      # 6272 nodes per core
RTOT = NCORES * NPC  # 50176 table rows
KE_CAP = 1152      # lo/hi packing capacity per (block, kind)
LO_LIM = 32768     # lo window rows [0, 32768)
HI_BASE = 17408    # hi window rows [17408, 50176)
# Aggregation unit: PAIR_BLOCKS=2 shares one dma_gather per window between
# two dst blocks (fewer Q7 fixed costs, but bigger gathers stall the SWDGE
# ring); PAIR_BLOCKS=1 gathers per block (measured faster on HW).
PAIR_BLOCKS = int(os.environ.get("GAT_PAIR", "1"))
NPAIR = (NB + PAIR_BLOCKS - 1) // PAIR_BLOCKS

EL01 = 384         # table elems/row layers 0/1: 256 h + 4 as + 4 ad + pad
EL2 = 128          # table elems/row layer 2: 64 h + 1 as + 1 ad + pad
BF = ml_dtypes.bfloat16

GBUFS = 6          # gather tile double-buffer depth
SINGLE_PACKET = os.environ.get("GAT_SP", "0") == "1"


def _a16(x):
    return (int(x) + 15) // 16 * 16


# ---------------- host preprocessing ----------------

def preprocess(edge_index):
    """Node->(core,block,slot) assignment, pair-packed edge arrays, and the
    static pair geometry consumed by build_kernel."""
    import heapq

    e0 = np.asarray(edge_index[0], np.int64)
    e1 = np.asarray(edge_index[1], np.int64)
    nonself = e0 != e1
    src = e0[nonself]
    dst = e1[nonself]
    # self-edge multiplicity: 1 (PyG added loop) + natural self edges
    mult = np.ones(N, np.int64)
    np.add.at(mult, e0[~nonself], 1)

    deg = np.bincount(dst, minlength=N)  # gather load per dst node

    nblocks = NCORES * NB
    order = np.argsort(-deg, kind="stable")
    heap = [(0, b) for b in range(nblocks)]
    heapq.heapify(heap)
    slots_used = np.zeros(nblocks, np.int64)
    node_block = np.empty(N, np.int64)
    node_slot = np.empty(N, np.int64)
    for n in order:
        while True:
            load, b = heapq.heappop(heap)
            if slots_used[b] < BS:
                break
        node_block[n] = b
        node_slot[n] = slots_used[b]
        slots_used[b] += 1
        heapq.heappush(heap, (load + int(deg[n]), b))

    row = node_block * BS + node_slot  # block-major global table row

    xperm = np.full(RTOT, -1, np.int64)
    xperm[row] = np.arange(N)

    erow = row[src]
    eblk = node_block[dst]
    eslot = node_slot[dst]

    order_e = np.argsort(eblk, kind="stable")
    bounds = np.searchsorted(eblk[order_e], np.arange(nblocks + 1))

    # pass 1: split lo/hi per block, count
    packed = {}
    cnt = np.zeros((NCORES, NB, 2), np.int64)
    for b in range(nblocks):
        c, bl = divmod(b, NB)
        es = order_e[bounds[b]:bounds[b + 1]]
        r_ = erow[es]
        dl = eslot[es]
        lo_f = r_ < HI_BASE
        hi_f = r_ >= LO_LIM
        flex = ~lo_f & ~hi_f
        n_lo = int(lo_f.sum())
        n_hi = int(hi_f.sum())
        n_fx = int(flex.sum())
        tot = n_lo + n_hi + n_fx
        assert tot <= 2 * KE_CAP, f"block {b} has {tot} edges > {2*KE_CAP}"
        add_lo = min(n_fx, max(0, min(KE_CAP, (tot + 1) // 2) - n_lo))
        if n_hi + (n_fx - add_lo) > KE_CAP:
            add_lo = n_fx - (KE_CAP - n_hi)
        assert 0 <= add_lo <= n_fx
        fx_idx = np.nonzero(flex)[0]
        sel_lo = np.zeros(len(es), bool)
        sel_lo[lo_f] = True
        sel_lo[fx_idx[:add_lo]] = True
        for kind, sel, base in ((0, sel_lo, 0), (1, ~sel_lo, HI_BASE)):
            rr = r_[sel]
            dd = dl[sel]
            o = np.argsort(rr, kind="stable")  # DMA locality
            packed[(b, kind)] = (rr[o] - base, dd[o])
            cnt[c, bl, kind] = len(rr)

    # ---- static pair geometry ----
    # pair p = blocks (2p, 2p+1); per (pair, kind): section sizes sa/sb
    # (16-aligned max over cores), gather reg = sa+sb, tiles tk.
    # Per block and kind: tile range [t0, t1) within the pair's tiles.
    geo = []
    for p in range(NPAIR):
        ba = PAIR_BLOCKS * p
        bb = ba + 1
        has_b = PAIR_BLOCKS == 2 and bb < NB
        pk = []
        for kind in range(2):
            sa = _a16(cnt[:, ba, kind].max())
            sb = _a16(cnt[:, bb, kind].max()) if has_b else 0
            skp = sa + sb
            tk = (skp + 127) // 128
            a0, a1 = 0, (sa + 127) // 128
            b0, b1 = sa // 128, tk
            pk.append(dict(sa=sa, sb=sb, skp=skp, tk=tk,
                           a=(a0, a1), b=(b0, b1) if has_b else None))
        geo.append(pk)

    # flat column offsets for idx16 / per-block dstc (dstcX: per block, both
    # kinds adjacent: [lo tiles | hi tiles] over that block's tile ranges)
    idx_off = []
    o = 0
    for p in range(NPAIR):
        idx_off.append(o)
        o += (geo[p][0]["tk"] * 128 + geo[p][1]["tk"] * 128) // 16
    idx_cols = o

    blk_off = []   # per block: column offset into dstc flat (tile units)
    blk_nt = []    # per block: (nt_lo, nt_hi)
    o = 0
    for bl in range(NB):
        p, x = divmod(bl, PAIR_BLOCKS)
        rng = [geo[p][k]["a" if x == 0 else "b"] for k in range(2)]
        nt = [r[1] - r[0] for r in rng]
        blk_off.append(o)
        blk_nt.append(tuple(nt))
        o += nt[0] + nt[1]
    dstc_cols = o

    dstr_cols = max((blk_nt[b][0] + blk_nt[b][1]) * 128 for b in range(NB))
    idx16 = np.full((NCORES, 128, idx_cols), -1, np.int16)
    dstc = np.full((NCORES, 128, dstc_cols), -1.0, np.float32)
    dstr = np.full((NCORES, NB, dstr_cols), -1.0, np.float32)
    maskc = np.zeros((NCORES, 128, NB), np.float32)
    mselfc = np.zeros((NCORES, 128, NB), np.float32)

    for c in range(NCORES):
        for p in range(NPAIR):
            ba = PAIR_BLOCKS * p
            bb = ba + 1
            has_b = PAIR_BLOCKS == 2 and bb < NB
            col0 = idx_off[p]
            for kind in range(2):
                gk = geo[p][kind]
                sa, sb, skp, tk = gk["sa"], gk["sb"], gk["skp"], gk["tk"]
                kep = tk * 128
                relA, ddA = packed[(c * NB + ba, kind)]
                full = np.full(kep, -1, np.int64)
                dloc = np.full(kep, -1.0, np.float32)
                kA = len(relA)
                full[:kA] = relA
                full[kA:sa] = 0
                dloc[:kA] = ddA.astype(np.float32)
                if has_b:
                    relB, ddB = packed[(c * NB + bb, kind)]
                    kB = len(relB)
                    full[sa:sa + kB] = relB
                    full[sa + kB:skp] = 0
                    dloc[sa:sa + kB] = ddB.astype(np.float32)
                w = full.reshape(kep // 16, 16).T.astype(np.int16)
                idx16[c, :, col0:col0 + kep // 16] = np.tile(w, (8, 1))
                col0 += kep // 16
                # per-block dstc over tile ranges (other block's slots = -1)
                for x, blk in ((0, ba),) + (((1, bb),) if has_b else ()):
                    t0, t1 = gk["a"] if x == 0 else gk["b"]
                    dl = dloc.copy()
                    if x == 0:
                        dl[sa:] = -1.0
                    else:
                        dl[:sa] = -1.0
                    seg = dl.reshape(tk, 128)[t0:t1]          # [nt, 128]
                    off = blk_off[blk] + (0 if kind == 0 else blk_nt[blk][0])
                    nt = t1 - t0
                    dstc[c, :, off:off + nt] = seg.T
                    dstr[c, blk, (0 if kind == 0 else
                                  blk_nt[blk][0] * 128):][:nt * 128] = \
                        seg.reshape(-1)

        for bl in range(NB):
            b = c * NB + bl
            used = slots_used[b]
            maskc[c, :used, bl] = 1.0
            nodes_b = np.where(node_block == b)[0]
            mselfc[c, node_slot[nodes_b], bl] = \
                mult[nodes_b].astype(np.float32)

    return dict(row=row, xperm=xperm, idx16=idx16, dstc=dstc, dstr=dstr,
                maskc=maskc, mselfc=mselfc, cnt=cnt, geo=geo,
                idx_off=idx_off, blk_off=blk_off, blk_nt=blk_nt,
                node_block=node_block, node_slot=node_slot)


def host_weights(inputs):
    """Extended weight matrices with folded attention vectors."""
    def ext(W, a_s, a_d, heads):
        Wh = W.reshape(W.shape[0], heads, HID)
        Was = np.einsum("khc,hc->kh", Wh, a_s)
        Wad = np.einsum("khc,hc->kh", Wh, a_d)
        return np.concatenate([W, Was, Wad], axis=1).astype(np.float32)

    W0e = ext(np.asarray(inputs["W0"], np.float32),
              np.asarray(inputs["a0s"], np.float32),
              np.asarray(inputs["a0d"], np.float32), HEADS)      # [128, 264]
    W1e = ext(np.asarray(inputs["W1"], np.float32),
              np.asarray(inputs["a1s"], np.float32),
              np.asarray(inputs["a1d"], np.float32), HEADS)      # [256, 264]
    W2e = ext(np.asarray(inputs["W2"], np.float32),
              np.asarray(inputs["a2s"], np.float32),
              np.asarray(inputs["a2d"], np.float32), 1)          # [256, 66]
    return W0e, W1e, W2e


def build_core_inputs(inputs, pp):
    """Per-core in_maps for run_bass_kernel_spmd."""
    x = np.asarray(inputs["x"], np.float32)
    W0e, W1e, W2e = host_weights(inputs)
    b0 = np.asarray(inputs["b0"], np.float32)
    b1 = np.asarray(inputs["b1"], np.float32)
    b2 = np.asarray(inputs["b2"], np.float32)

    iota_row = np.tile(np.arange(128, dtype=np.float32), (128, 1))
    iota_col = np.arange(128, dtype=np.float32).reshape(128, 1)
    ones1 = np.ones((1, 128), np.float32)
    ident = np.eye(128, dtype=np.float32)

    consts = dict(
        w0e=W0e.astype(BF),
        w1e=W1e.reshape(2, 128, F1 + 2 * HEADS).astype(BF),
        w2e=W2e.reshape(2, 128, HID + 2).astype(BF),
        b0r=np.tile(b0, (128, 1)).astype(BF),
        b1r=np.tile(b1, (128, 1)).astype(BF),
        b2r=np.tile(b2, (128, 1)).astype(BF),
        iota_row=iota_row.astype(BF), iota_col=iota_col.astype(BF),
        ones1=ones1.astype(BF), ident=ident.astype(BF),
    )

    in_maps = []
    for c in range(NCORES):
        xtb = np.zeros((NB, IN_C, BS), np.float32)
        rows = np.arange(c * NPC, (c + 1) * NPC)
        nodes = pp["xperm"][rows].reshape(NB, BS)
        for b in range(NB):
            nb = nodes[b]
            valid = nb >= 0
            if valid.any():
                xtb[b][:, valid] = x[nb[valid]].T
        m = dict(
            xtb=xtb.astype(BF),
            idx16=pp["idx16"][c],
            dstc=pp["dstc"][c].astype(BF),
            dstr=pp["dstr"][c].astype(BF),
            maskc=pp["maskc"][c].astype(BF),
            mselfc=pp["mselfc"][c].astype(BF),
            **consts,
        )
        in_maps.append(m)
    return in_maps


# ---------------- numpy emulation of the device data path ----------------

def _emulate_layer(tables_in, pp, We, bias, heads, Fo, relu, el):
    """tables_in: node-major feature mat [RTOT, F_in] (f32).
    Mirrors the pair-packed device data path (reads pp's flat arrays)."""
    geo = pp["geo"]
    ncols = Fo + 2 * heads
    tb = (tables_in.astype(BF).astype(np.float32)
          @ We.astype(BF).astype(np.float32))
    table = np.zeros((RTOT, el), BF)
    table[:, :ncols] = tb.astype(BF)
    as_all = tb[:, Fo:Fo + heads].astype(BF).astype(np.float32)
    ad_all = tb[:, Fo + heads:Fo + 2 * heads].astype(BF).astype(np.float32)

    def lrexp(z):
        return np.exp(np.maximum(z, 0.2 * z)).astype(np.float32)

    out = np.zeros((RTOT, Fo), np.float32)
    for c in range(NCORES):
        for p in range(len(geo)):
            col0 = pp["idx_off"][p]
            gs = []
            dls = []
            for kind in range(2):
                gk = geo[p][kind]
                kep = gk["tk"] * 128
                w = pp["idx16"][c][:16, col0:col0 + kep // 16]
                col0 += kep // 16
                rel = w.T.reshape(-1).astype(np.int64)
                r = gk["skp"]
                base = 0 if kind == 0 else HI_BASE
                rows = rel[:r] + base
                g = np.zeros((kep, el), np.float32)
                g[:r] = np.asarray(table[rows], np.float32)
                gs.append(g)
            ba = PAIR_BLOCKS * p
            blks = ((0, ba),) + (((1, ba + 1),)
                                 if PAIR_BLOCKS == 2 and ba + 1 < NB else ())
            for x, blk in blks:
                bl = blk
                rbase = c * NPC + bl * BS
                agg = np.zeros((BS, Fo), np.float32)
                den = np.zeros((BS, heads), np.float32)
                for kind in range(2):
                    gk = geo[p][kind]
                    t0, t1 = gk["a"] if x == 0 else gk["b"]
                    off = pp["blk_off"][bl] + (
                        0 if kind == 0 else pp["blk_nt"][bl][0])
                    nt = t1 - t0
                    dl = pp["dstc"][c][:, off:off + nt].T.reshape(-1)
                    dl = dl.astype(np.int64)                    # [nt*128]
                    g = gs[kind][t0 * 128:t1 * 128]             # [nt*128, el]
                    valid = dl >= 0
                    a_s = g[:, Fo:Fo + heads]
                    a_d = np.where(valid[:, None],
                                   ad_all[rbase + dl % 128 if False else
                                          rbase + np.clip(dl, 0, None)], 0.0)
                    s = lrexp(a_s + a_d).astype(BF).astype(np.float32)
                    s = np.where(valid[:, None], s, 0.0)
                    hsc = (g[:, :Fo].reshape(-1, heads, HID)
                           * s[:, :, None]).astype(BF).astype(np.float32)
                    hsc = hsc.reshape(-1, Fo)
                    np.add.at(agg, np.clip(dl, 0, None)[valid], hsc[valid])
                    np.add.at(den, np.clip(dl, 0, None)[valid], s[valid])
                # self loops
                ms = pp["mselfc"][c][:, bl]
                ss = lrexp(as_all[rbase:rbase + BS]
                           + ad_all[rbase:rbase + BS])
                se = (ss * ms[:, None]).astype(BF).astype(np.float32)
                h_own = np.asarray(table[rbase:rbase + BS, :Fo], np.float32)
                hs = (h_own.reshape(BS, heads, HID)
                      * se[:, :, None]).astype(BF).astype(np.float32)
                agg += hs.reshape(BS, Fo)
                den += se
                o = agg.reshape(BS, heads, HID) / (den + 1e-16)[:, :, None]
                o = o.reshape(BS, Fo) + bias
                if relu:
                    o = np.maximum(o, 0.0)
                out[rbase:rbase + BS] = o
    return out


def emulate(inputs, pp=None):
    """Full numpy emulation; returns [1, OUT_C]."""
    if pp is None:
        pp = preprocess(np.asarray(inputs["edge_index"]))
    x = np.asarray(inputs["x"], np.float32)
    W0e, W1e, W2e = host_weights(inputs)
    h = np.zeros((RTOT, IN_C), np.float32)
    valid = pp["xperm"] >= 0
    h[valid] = x[pp["xperm"][valid]]

    b0 = np.asarray(inputs["b0"], np.float32)
    b1 = np.asarray(inputs["b1"], np.float32)
    b2 = np.asarray(inputs["b2"], np.float32)

    h0 = _emulate_layer(h, pp, W0e, b0, HEADS, F1, True, EL01)
    h1 = _emulate_layer(h0, pp, W1e, b1, HEADS, F1, True, EL01)
    h2 = _emulate_layer(h1, pp, W2e, b2, 1, HID, False, EL2)

    g = h2[valid].sum(axis=0, keepdims=True) / N
    return (g @ np.asarray(inputs["hw"], np.float32)
            + np.asarray(inputs["hb"], np.float32)).astype(np.float32)


# ---------------- device kernel ----------------

_BUILT = None
_BUILT_KEY = None


def build_kernel(geo, idx_off, blk_off, blk_nt, idx_cols, dstc_cols,
                 dstr_cols):
    import concourse.bacc as bacc
    import concourse.mybir as mybir
    import concourse.tile as tile
    from concourse import library_config

    f32 = mybir.dt.float32
    bf16 = mybir.dt.bfloat16
    i16 = mybir.dt.int16
    Alu = mybir.AluOpType
    Act = mybir.ActivationFunctionType

    nc = bacc.Bacc("TRN2", target_bir_lowering=False, debug=False,
                   num_devices=NCORES, num_swdge_queues=4)

    max_tk = max(g[k]["tk"] for g in geo for k in range(2))
    max_nt = max(nt[0] + nt[1] for nt in blk_nt)

    # ---- I/O ----
    xtb_d = nc.dram_tensor("xtb", [NB, IN_C, BS], bf16, kind="ExternalInput")
    idx16_d = nc.dram_tensor("idx16", [128, idx_cols], i16,
                             kind="ExternalInput")
    dstc_d = nc.dram_tensor("dstc", [128, dstc_cols], bf16,
                            kind="ExternalInput")
    dstr_d = nc.dram_tensor("dstr", [NB, dstr_cols], bf16,
                            kind="ExternalInput")
    maskc_d = nc.dram_tensor("maskc", [128, NB], bf16, kind="ExternalInput")
    mselfc_d = nc.dram_tensor("mselfc", [128, NB], bf16, kind="ExternalInput")
    w0e_d = nc.dram_tensor("w0e", [IN_C, F1 + 2 * HEADS], bf16,
                           kind="ExternalInput")
    w1e_d = nc.dram_tensor("w1e", [2, 128, F1 + 2 * HEADS], bf16,
                           kind="ExternalInput")
    w2e_d = nc.dram_tensor("w2e", [2, 128, HID + 2], bf16,
                           kind="ExternalInput")
    b0r_d = nc.dram_tensor("b0r", [128, F1], bf16, kind="ExternalInput")
    b1r_d = nc.dram_tensor("b1r", [128, F1], bf16, kind="ExternalInput")
    b2r_d = nc.dram_tensor("b2r", [128, HID], bf16, kind="ExternalInput")
    iota_row_d = nc.dram_tensor("iota_row", [128, 128], bf16,
                                kind="ExternalInput")
    iota_col_d = nc.dram_tensor("iota_col", [128, 1], bf16,
                                kind="ExternalInput")
    ones1_d = nc.dram_tensor("ones1", [1, 128], bf16, kind="ExternalInput")
    ident_d = nc.dram_tensor("ident", [128, 128], bf16, kind="ExternalInput")
    out_d = nc.dram_tensor("out_part", [1, OUT_C], f32, kind="ExternalOutput")

    # internal DRAM
    tables = []
    shards = []
    for li, el in enumerate([EL01, EL01, EL2]):
        tables.append(nc.dram_tensor(f"table{li}", [RTOT, el], bf16,
                                     addr_space="Shared"))
        shards.append(nc.dram_tensor(f"shard{li}", [NPC, el], bf16))

    rg = [list(range(NCORES))]

    with tile.TileContext(nc) as tc:
        with (
            tc.tile_pool(name="const", bufs=1) as cpool,
            tc.tile_pool(name="gather", bufs=GBUFS) as gpool,
            tc.tile_pool(name="onehot", bufs=4) as mpool,
            tc.tile_pool(name="work", bufs=3) as wpool,
            tc.tile_pool(name="small", bufs=4) as spool,
            tc.tile_pool(name="adas", bufs=1) as apool,
            tc.tile_pool(name="ps_agg", bufs=2, space="PSUM") as ppagg,
            tc.tile_pool(name="ps_pad", bufs=2, space="PSUM") as pppad,
            tc.tile_pool(name="ps_rep", bufs=1, space="PSUM") as pprep,
            tc.tile_pool(name="ps_tp", bufs=1, space="PSUM") as pptp,
            tc.tile_pool(name="ps_tf", bufs=1, space="PSUM") as pptf,
            tc.tile_pool(name="ps_sum", bufs=1, space="PSUM") as ppsum,
        ):
            def load_const(tag, dram, shape, dtype=bf16, view=None):
                t = cpool.tile(shape, dtype, tag=tag)
                nc.sync.dma_start(out=t[:], in_=view if view is not None
                                  else dram[:])
                return t

            w0e_s = load_const("w0e", w0e_d, [IN_C, F1 + 2 * HEADS])
            w1e_s = load_const("w1e", w1e_d, [128, 2, F1 + 2 * HEADS],
                               view=w1e_d[:].rearrange("c p j -> p c j"))
            w2e_s = load_const("w2e", w2e_d, [128, 2, HID + 2],
                               view=w2e_d[:].rearrange("c p j -> p c j"))
            b0r_s = load_const("b0r", b0r_d, [128, F1])
            b1r_s = load_const("b1r", b1r_d, [128, F1])
            b2r_s = load_const("b2r", b2r_d, [128, HID])
            iota_row_s = load_const("iota_row", iota_row_d, [128, 128])
            iota_col_s = load_const("iota_col", iota_col_d, [128, 1])
            ones1_s = load_const("ones1", ones1_d, [1, 128])
            ident_s = load_const("ident", ident_d, [128, 128])
            idx16_s = load_const("idx16", idx16_d, [128, idx_cols], i16)
            dstc_s = load_const("dstc", dstc_d, [128, dstc_cols])
            maskc_s = load_const("maskc", maskc_d, [128, NB])
            mselfc_s = load_const("mselfc", mselfc_d, [128, NB])

            nc.gpsimd.load_library(library_config.mlp)

            # persistent per-layer alpha tiles [128, NB*heads]
            as_all0 = apool.tile([128, NB * HEADS], bf16, tag="as0")
            as_all1 = apool.tile([128, NB * HEADS], bf16, tag="as1")
            as_all2 = apool.tile([128, NB], bf16, tag="as2")
            ad_all0 = apool.tile([128, NB * HEADS], bf16, tag="ad0")
            ad_all1 = apool.tile([128, NB * HEADS], bf16, tag="ad1")
            ad_all2 = apool.tile([128, NB], bf16, tag="ad2")
            as_all = [as_all0, as_all1, as_all2]
            ad_all = [ad_all0, ad_all1, ad_all2]

            LCFG = [  # heads, Fo, ncols, el, bias, relu
                (HEADS, F1, F1 + 2 * HEADS, EL01, b0r_s, True),
                (HEADS, F1, F1 + 2 * HEADS, EL01, b1r_s, True),
                (1, HID, HID + 2, EL2, b2r_s, False),
            ]

            def transform_block(layer, b, lhsT0, lhsT1):
                heads, Fo, ncols, el, _bias, _relu = LCFG[layer]
                shard = shards[layer]
                ps = pptf.tile([128, 512], f32, tag="tf", space="PSUM")
                if layer == 0:
                    nc.tensor.matmul(out=ps[:, :ncols], lhsT=lhsT0,
                                     rhs=w0e_s[:], start=True, stop=True)
                else:
                    we = w1e_s if layer == 1 else w2e_s
                    nc.tensor.matmul(out=ps[:, :ncols], lhsT=lhsT0,
                                     rhs=we[:, 0, :], start=True, stop=False)
                    nc.tensor.matmul(out=ps[:, :ncols], lhsT=lhsT1,
                                     rhs=we[:, 1, :], start=False, stop=True)
                tb = wpool.tile([128, EL01], bf16, tag="tb")
                nc.vector.tensor_copy(out=tb[:, :ncols], in_=ps[:, :ncols])
                nc.vector.tensor_copy(
                    out=as_all[layer][:, b * heads:(b + 1) * heads],
                    in_=ps[:, Fo:Fo + heads])
                nc.vector.tensor_copy(
                    out=ad_all[layer][:, b * heads:(b + 1) * heads],
                    in_=ps[:, Fo + heads:Fo + 2 * heads])
                nc.sync.dma_start(out=shard[b * BS:(b + 1) * BS, :],
                                  in_=tb[:, :el])

            def allgather(layer):
                nc.gpsimd.collective_compute(
                    "AllGather", mybir.AluOpType.bypass,
                    replica_groups=rg, ins=[shards[layer][:].opt()],
                    outs=[tables[layer][:].opt()])

            def agg_pair(layer, p):
                """Aggregate blocks (2p, 2p+1); returns per-block results."""
                heads, Fo, ncols, el, bias, relu = LCFG[layer]
                table = tables[layer]
                shard = shards[layer]
                views = [table[0:LO_LIM, :], table[HI_BASE:HI_BASE + 32768, :]]
                gA, gB = geo[p][0], geo[p][1]
                tkL, tkH = gA["tk"], gB["tk"]
                ntt = tkL + tkH
                ba = PAIR_BLOCKS * p
                blocks = [(0, ba)] + ([(1, ba + 1)]
                                      if PAIR_BLOCKS == 2 and ba + 1 < NB
                                      else [])

                # paired gathers (critical Q7 stream)
                gtiles = []
                col0 = idx_off[p]
                for kind in range(2):
                    gk = geo[p][kind]
                    kep = gk["tk"] * 128
                    g = gpool.tile([128, gk["tk"], el], bf16, tag="g")
                    nc.gpsimd.dma_gather(
                        g[:], views[kind],
                        idx16_s[:, col0:col0 + kep // 16],
                        kep, gk["skp"], el,
                        single_packet=SINGLE_PACKET,
                        queue_num=2 * (p % 2) + kind)
                    col0 += kep // 16
                    gtiles.append(g)

                # per-block one-hot M/MT + adp into the shared pair pad_
                pad_ = pppad.tile([128, ntt * heads], f32, tag="adp",
                                  space="PSUM")
                Ms = {}

                def tile_writers(gt):
                    """Blocks covering pair-tile gt (for adp start/stop)."""
                    kind = 0 if gt < tkL else 1
                    t = gt - (0 if kind == 0 else tkL)
                    gk = geo[p][kind]
                    ws = []
                    for x, blk in blocks:
                        rng = gk["a"] if x == 0 else gk["b"]
                        if rng[0] <= t < rng[1]:
                            ws.append(x)
                    return ws

                for x, blk in blocks:
                    nt_lo, nt_hi = blk_nt[blk]
                    ntb = nt_lo + nt_hi
                    off = blk_off[blk]
                    M = mpool.tile([128, max_nt, 128], bf16, tag="M")
                    nc.vector.tensor_tensor(
                        out=M[:, :ntb, :],
                        in0=dstc_s[:, off:off + ntb].unsqueeze(-1)
                            .broadcast_to([128, ntb, 128]),
                        in1=iota_row_s[:].unsqueeze(1)
                            .broadcast_to([128, ntb, 128]),
                        op=Alu.is_equal)
                    Ms[x] = (M, ntb)
                    MT = mpool.tile([128, max_nt * 128], bf16, tag="MT")
                    dr = spool.tile([1, max_nt * 128], bf16, tag="dr")
                    nc.sync.dma_start(out=dr[:, :ntb * 128],
                                      in_=dstr_d[blk:blk + 1, :ntb * 128])
                    for o in range(0, ntb * 128, 512):
                        wd = min(512, ntb * 128 - o)
                        pr = pprep.tile([128, 512], f32, tag="rep",
                                        space="PSUM")
                        nc.tensor.matmul(out=pr[:, :wd], lhsT=ones1_s[:],
                                         rhs=dr[:, o:o + wd],
                                         start=True, stop=True)
                        nc.vector.tensor_tensor(
                            out=MT[:, o:o + wd], in0=pr[:, :wd],
                            in1=iota_col_s[:].broadcast_to([128, wd]),
                            op=Alu.is_equal)
                    # adp matmuls over this block's tiles
                    for kind in range(2):
                        gk = geo[p][kind]
                        t0, t1 = gk["a"] if x == 0 else gk["b"]
                        jbase = 0 if kind == 0 else nt_lo
                        gtb = 0 if kind == 0 else tkL
                        for t in range(t0, t1):
                            gt = gtb + t
                            ws = tile_writers(gt)
                            nc.tensor.matmul(
                                out=pad_[:, gt * heads:(gt + 1) * heads],
                                lhsT=MT[:, (jbase + t - t0) * 128:
                                        (jbase + t - t0 + 1) * 128],
                                rhs=ad_all[layer][:,
                                                  blk * heads:
                                                  (blk + 1) * heads],
                                start=(ws[0] == x), stop=(ws[-1] == x))

                # z for all pair tiles + self-z tails (one group per block)
                nzc = ntt * heads
                nself = len(blocks) * heads
                z = spool.tile([128, nzc + nself], f32, tag="z")
                for kind in range(2):
                    gk = geo[p][kind]
                    zof = (0 if kind == 0 else tkL) * heads
                    nc.vector.tensor_tensor(
                        out=z[:, zof:zof + gk["tk"] * heads]
                            .rearrange("p (t h) -> p t h", t=gk["tk"]),
                        in0=gtiles[kind][:, :, Fo:Fo + heads],
                        in1=pad_[:, zof:zof + gk["tk"] * heads]
                            .rearrange("p (t h) -> p t h", t=gk["tk"]),
                        op=Alu.add)
                for x, blk in blocks:
                    zof = nzc + x * heads
                    nc.vector.tensor_tensor(
                        out=z[:, zof:zof + heads],
                        in0=as_all[layer][:, blk * heads:(blk + 1) * heads],
                        in1=ad_all[layer][:, blk * heads:(blk + 1) * heads],
                        op=Alu.add)
                zl = spool.tile([128, nzc + nself], f32, tag="zl")
                nc.vector.scalar_tensor_tensor(
                    out=zl[:], in0=z[:], scalar=0.2, in1=z[:],
                    op0=Alu.mult, op1=Alu.max)
                tmpS = wpool.tile([128, ntt, Fo + heads], bf16, tag="tmpS")
                nc.scalar.activation(
                    tmpS[:, :, Fo:Fo + heads],
                    zl[:, :nzc].rearrange("p (t h) -> p t h", t=ntt),
                    Act.Exp)
                ses = spool.tile([128, nself], f32, tag="ses")
                nc.scalar.activation(ses[:], zl[:, nzc:nzc + nself], Act.Exp)
                sv = tmpS[:, :, Fo:Fo + heads]
                for kind in range(2):
                    gk = geo[p][kind]
                    ts0 = 0 if kind == 0 else tkL
                    for hh in range(heads):
                        nc.vector.tensor_tensor(
                            out=tmpS[:, ts0:ts0 + gk["tk"],
                                     hh * HID:(hh + 1) * HID],
                            in0=gtiles[kind][:, :, hh * HID:(hh + 1) * HID],
                            in1=sv[:, ts0:ts0 + gk["tk"], hh:hh + 1]
                                .broadcast_to([128, gk["tk"], HID]),
                            op=Alu.mult)

                # per-block fused (agg | den) + epilogue
                results = []
                for x, blk in blocks:
                    M, ntb = Ms[x]
                    nt_lo, _nt_hi = blk_nt[blk]
                    pagg = ppagg.tile([128, Fo + heads], f32, tag="agg",
                                      space="PSUM")
                    mms = []
                    for kind in range(2):
                        gk = geo[p][kind]
                        t0, t1 = gk["a"] if x == 0 else gk["b"]
                        jbase = 0 if kind == 0 else nt_lo
                        gtb = 0 if kind == 0 else tkL
                        for t in range(t0, t1):
                            mms.append((jbase + t - t0, gtb + t))
                    for i, (j, gt) in enumerate(mms):
                        nc.tensor.matmul(
                            out=pagg[:],
                            lhsT=M[:, j, :],
                            rhs=tmpS[:, gt, :],
                            start=(i == 0), stop=(i == len(mms) - 1))

                    se = spool.tile([128, heads], bf16, tag="se")
                    nc.vector.tensor_tensor(
                        out=se[:], in0=ses[:, x * heads:(x + 1) * heads],
                        in1=mselfc_s[:, blk:blk + 1]
                            .broadcast_to([128, heads]),
                        op=Alu.mult)
                    h_own = wpool.tile([128, Fo], bf16, tag="hown")
                    nc.sync.dma_start(out=h_own[:],
                                      in_=shard[blk * BS:(blk + 1) * BS, :Fo])
                    hs = wpool.tile([128, Fo + heads], bf16, tag="hs")
                    for hh in range(heads):
                        nc.vector.tensor_tensor(
                            out=hs[:, hh * HID:(hh + 1) * HID],
                            in0=h_own[:, hh * HID:(hh + 1) * HID],
                            in1=se[:, hh:hh + 1].broadcast_to([128, HID]),
                            op=Alu.mult)
                    nc.vector.tensor_copy(out=hs[:, Fo:Fo + heads], in_=se[:])

                    t1_ = wpool.tile([128, Fo + heads], f32, tag="t1")
                    nc.vector.tensor_tensor(out=t1_[:], in0=pagg[:],
                                            in1=hs[:], op=Alu.add)
                    den = spool.tile([128, heads], f32, tag="den")
                    nc.vector.tensor_scalar(out=den[:],
                                            in0=t1_[:, Fo:Fo + heads],
                                            scalar1=1e-16, scalar2=None,
                                            op0=Alu.add)
                    rec = spool.tile([128, heads], f32, tag="rec")
                    nc.vector.reciprocal(out=rec[:], in_=den[:])
                    o1 = wpool.tile([128, Fo], f32, tag="o1")
                    nc.vector.tensor_tensor(
                        out=o1[:].rearrange("p (h f) -> p h f", h=heads),
                        in0=t1_[:, :Fo].rearrange("p (h f) -> p h f",
                                                  h=heads),
                        in1=rec[:].unsqueeze(-1)
                            .broadcast_to([128, heads, HID]),
                        op=Alu.mult)
                    o2 = wpool.tile([128, Fo], bf16, tag="o2")
                    nc.vector.tensor_tensor(out=o2[:], in0=o1[:],
                                            in1=bias[:, :Fo], op=Alu.add)
                    if layer == 2:
                        results.append((blk, o2))
                        continue
                    o3 = wpool.tile([128, Fo], bf16, tag="o3")
                    nc.scalar.activation(o3[:], o2[:], Act.Relu)
                    hTb = []
                    for k2 in range(2):
                        pt = pptp.tile([128, 128], bf16, tag="tp",
                                       space="PSUM")
                        nc.tensor.transpose(pt[:],
                                            o3[:, k2 * 128:(k2 + 1) * 128],
                                            ident_s[:])
                        ht = spool.tile([128, 128], bf16, tag=f"ht{k2}")
                        nc.vector.tensor_copy(out=ht[:], in_=pt[:])
                        hTb.append(ht)
                    results.append((blk, hTb))
                return results

            # ---- layer 0 transform (batched x^T loads) ----
            for b0_ in range(0, NB, 4):
                nbk = min(4, NB - b0_)
                xb = wpool.tile([IN_C, 4 * BS], bf16, tag="xtb")
                nc.sync.dma_start(
                    out=xb[:, :nbk * BS].rearrange("c (b s) -> c b s", b=nbk),
                    in_=xtb_d[b0_:b0_ + nbk].rearrange("b c s -> c b s"))
                for j in range(nbk):
                    transform_block(0, b0_ + j,
                                    xb[:, j * BS:(j + 1) * BS], None)
            allgather(0)

            # prime gather tiles (stale-tail rows must be finite)
            for _ in range(GBUFS):
                g = gpool.tile([128, max_tk, EL01], bf16, tag="g")
                nc.vector.memset(g[:], 0.0)

            # ---- layer 0/1: aggregate + interleaved next transform ----
            for layer in range(2):
                for p in range(NPAIR):
                    for blk, hTb in agg_pair(layer, p):
                        transform_block(layer + 1, blk, hTb[0][:], hTb[1][:])
                allgather(layer + 1)

            # ---- layer 2: aggregate + masked column sum ----
            psum_sum = ppsum.tile([1, OUT_C], f32, tag="sum", space="PSUM")
            first = True
            for p in range(NPAIR):
                for blk, o2 in agg_pair(2, p):
                    nc.tensor.matmul(out=psum_sum[:],
                                     lhsT=maskc_s[:, blk:blk + 1],
                                     rhs=o2[:], start=first,
                                     stop=(blk == NB - 1))
                    first = False
            osb = spool.tile([1, OUT_C], f32, tag="osb")
            nc.vector.tensor_copy(out=osb[:], in_=psum_sum[:])
            nc.sync.dma_start(out=out_d[:], in_=osb[:])

    nc.compile()
    return nc


def _geo_key(pp):
    return repr([(g[0]["sa"], g[0]["sb"], g[1]["sa"], g[1]["sb"])
                 for g in pp["geo"]])


def _get_built(pp):
    global _BUILT, _BUILT_KEY
    key = _geo_key(pp)
    if _BUILT is None or _BUILT_KEY != key:
        _BUILT = build_kernel(pp["geo"], pp["idx_off"], pp["blk_off"],
                              pp["blk_nt"], pp["idx16"].shape[2],
                              pp["dstc"].shape[2], pp["dstr"].shape[2])
        _BUILT_KEY = key
    return _BUILT


def kernel(**inputs) -> np.ndarray:
    from concourse.bass_utils import run_bass_kernel_spmd

    pp = preprocess(np.asarray(inputs["edge_index"]))
    in_maps = build_core_inputs(inputs, pp)
    nc = _get_built(pp)
    res = run_bass_kernel_spmd(nc, in_maps, core_ids=list(range(NCORES)))
    parts = np.stack([r["out_part"][0] for r in res.results])  # [8, 64]
    g = parts.sum(axis=0, keepdims=True) / N
    out = (g @ np.asarray(inputs["hw"], np.float32)
           + np.asarray(inputs["hb"], np.float32)).astype(np.float32)
    return out
